# revision 1
# baseline (speedup 1.0000x reference)
"""NeuroSAT message-passing GNN on 8 TRN2 NeuronCores (Bass/Tile).

Sharding: clause dim sharded 8-way (2048 padded clauses/core); literal dim
permuted so core i owns problem i's 500 vars (+12 pads) as 1024 lit rows
(512 pos + 512 neg).  Per round (pipelined):
  GEMM2 groups 0,1 -> RS half0 ; groups 2,3 -> RS half1 (bf16 ReduceScatter)
  L-LSTM + L_pre MLP per half, AllGather halves of L_pre (bf16)
  GEMM1 LC.T = L_pre.T @ B1 ; C-LSTM ; C_pre MLP ; repeat
M (counts) is exact in fp8e4m3; fp8 M blocks stream from HBM as the moving
operand against bf16 stationary activations.  Zero-contribution dummy
matmuls keep the PE HAM-warm across collective waits.
"""

import numpy as np
import ml_dtypes

import concourse.bass as bass
import concourse.bacc as bacc
import concourse.mybir as mybir
import concourse.tile as tile
from concourse import bass_utils

F32 = mybir.dt.float32
BF16 = mybir.dt.bfloat16
FP8 = mybir.dt.float8e4
AF = mybir.ActivationFunctionType

N_CORES = 8
DIM = 128
N_ROUNDS = 16
N_VARS = 4000
VPC = 500            # real vars per core (= vars per problem)
VPAD = 512           # padded vars per core
LL = 2 * VPAD        # 1024 lit rows per core
LPAD = N_CORES * LL  # 8192
CC = 2048            # padded clauses per core
CPAD = N_CORES * CC  # 16384
KL = LPAD // 128     # 64 k-tiles over lits
KC = CC // 128       # 16 k-tiles over clauses

# GEMM2 groups: group g computes 512-lit chunks J_SETS[g]; chunk j covers
# local lit rows [512*(j%2)...) of destination core j//2.  Groups 0,1 cover
# all even j (RS half 0 = every core's rows 0:512); groups 2,3 odd j.
J_SETS = [[0, 2, 4, 6], [8, 10, 12, 14], [1, 3, 5, 7], [9, 11, 13, 15]]

N_WARM1 = 0          # dummy MMs per gate group, L half 0 (RS_0 wait)
N_WARM_G1 = 48       # dummy MM prefix on GEMM1 (AG + load window)
N_WARM2 = 10         # dummy MMs per gate group, L half 1 (RS_1 wait)

nbf = ml_dtypes.bfloat16
nf8 = ml_dtypes.float8_e4m3

_CACHE = {}


def _build():
    """Build + compile the SPMD program once (shape-only, no input values)."""
    if "nc" in _CACHE:
        return _CACHE["nc"]

    nc = bacc.Bacc("TRN2", target_bir_lowering=False, debug=False,
                   num_devices=N_CORES)

    def din(name, shape, dt):
        return nc.dram_tensor(name, shape, dt, kind="ExternalInput")

    # b1: 16 packed groups of 4 k-tiles; rows ordered [half h, core c, r<512]
    b1 = din("b1", [KL // 4, DIM, 4 * CC], FP8)
    # b2[g]: group g's 16 k-tiles packed 4-per-DMA: [4 groups, 4 qgrp, 128, 4*2048]
    b2 = din("b2", [4, 4, DIM, 4 * 2048], FP8)
    lh0t = din("lh0t", [DIM, LL], BF16)
    ch0t = din("ch0t", [DIM, CC], BF16)
    id128 = din("id128", [DIM, DIM], BF16)

    w = {}
    for p in ("lmsg", "cmsg", "lvote"):
        for i in (1, 2, 3):
            shp = [DIM, 1] if (p == "lvote" and i == 3) else [DIM, DIM]
            w[f"{p}_w{i}t"] = din(f"{p}_w{i}t", shp, BF16)
            bshp = [1, 1] if (p == "lvote" and i == 3) else [DIM, 1]
            w[f"{p}_b{i}"] = din(f"{p}_b{i}", bshp, F32)
    w["cu_wiht"] = din("cu_wiht", [DIM, 4 * DIM], BF16)
    w["cu_whht"] = din("cu_whht", [DIM, 4 * DIM], BF16)
    w["lu_wiht_cl"] = din("lu_wiht_cl", [DIM, 4 * DIM], BF16)
    w["lu_wiht_fl"] = din("lu_wiht_fl", [DIM, 4 * DIM], BF16)
    w["lu_whht"] = din("lu_whht", [DIM, 4 * DIM], BF16)
    cu_bias_d = din("cu_bias", [4, DIM], F32)
    lu_bias_d = din("lu_bias", [4, DIM], F32)

    vote_out = nc.dram_tensor("vote", [1, LL], F32, kind="ExternalOutput")

    with tile.TileContext(nc) as tc, \
         tc.tile_pool(name="const", bufs=1) as const, \
         tc.tile_pool(name="sb", bufs=2) as sb, \
         tc.tile_pool(name="sb3", bufs=2) as sb3, \
         tc.tile_pool(name="ps", bufs=6, space="PSUM") as ps, \
         tc.tile_pool(name="pstr", bufs=2, space="PSUM") as pstr, \
         tc.tile_pool(name="dram", bufs=2, space="DRAM") as dram:

        # ---- resident b2: first 12 of 16 blocks of [128, 8192] fp8
        N_B2_RES = 11
        b2r = const.tile([DIM, N_B2_RES * 4 * 2048], FP8, tag="b2r")
        for g in range(4):
            for q in range(4):
                if g * 4 + q >= N_B2_RES:
                    continue
                sl = slice((g * 4 + q) * 8192, (g * 4 + q + 1) * 8192)
                nc.sync.dma_start(b2r[:, sl], b2.ap()[g, q, :, :])

        # ---- load constants/weights into SBUF
        cw = {}
        for k in w:
            t = const.tile(list(w[k].shape), w[k].dtype, tag=f"cw_{k}")
            nc.sync.dma_start(t[:], w[k].ap())
            cw[k] = t
        for k, dte in (("cu_bias", cu_bias_d), ("lu_bias", lu_bias_d)):
            t = const.tile([DIM, 4], F32, tag=f"cw_{k}")
            nc.sync.dma_start(t[:], dte.ap().rearrange("g p -> p g"))
            cw[k] = t
        idt = const.tile([DIM, DIM], BF16, tag="idt")
        nc.sync.dma_start(idt[:], id128.ap())
        zbf = const.tile([DIM, 512], BF16, tag="zbf")
        nc.vector.memset(zbf[:], 0.0)

        # ---- persistent state (feature-major)
        lht = const.tile([DIM, LL], BF16, tag="lht")
        lct = const.tile([DIM, LL], F32, tag="lct")
        cht = const.tile([DIM, CC], BF16, tag="cht")
        cct = const.tile([DIM, CC], F32, tag="cct")
        nc.sync.dma_start(lht[:], lh0t.ap())
        nc.sync.dma_start(cht[:], ch0t.ap())
        nc.vector.memset(lct[:], 0.0)
        nc.vector.memset(cct[:], 0.0)

        def dma2(dst, src):
            """Split a [128, N] transfer across two DMA queues by partitions."""
            nc.sync.dma_start(dst[0:64, :], src[0:64, :])
            nc.sync.dma_start(dst[64:DIM, :], src[64:DIM, :])

        def mlp_chunk(x, pfx, sl, n, out_dt=BF16, tagsfx=""):
            """3-layer MLP on columns sl (chunks of <=512) of x [128, *]."""
            cur = x
            for li in (1, 2, 3):
                wt = cw[f"{pfx}_w{li}t"]
                bt = cw[f"{pfx}_b{li}"]
                m = wt.shape[1]
                o = sb.tile([m, n], out_dt if li == 3 else BF16, bufs=1,
                            tag=f"{pfx}_h{li}{tagsfx}", name=f"{pfx}_h{li}{tagsfx}")
                for rc in range(n // 512):
                    c0 = rc * 512
                    pt = ps.tile([m, 512], F32, tag="ps", name="mlp_ps")
                    src = cur[:, sl.start + c0:sl.start + c0 + 512] if li == 1 \
                        else cur[:, c0:c0 + 512]
                    nc.tensor.matmul(pt[:], wt[:], src, start=True, stop=True)
                    func = AF.Relu if li < 3 else AF.Identity
                    nc.scalar.activation(o[:, c0:c0 + 512], pt[:], func,
                                         bias=bt[:, 0:1])
                cur = o
            return cur

        def lstm_elementwise(gps, bias, c_st, h_st, rc0, n):
            """gps: 4 psum tiles [128, n] (i,f,g,o); updates states [:, rc0:rc0+n]."""
            sl = slice(rc0, rc0 + n)
            sig_i = sb.tile([DIM, n], BF16, tag="lw_si", bufs=1, name="sig_i")
            sig_f = sb.tile([DIM, n], BF16, tag="lw_sf", bufs=1, name="sig_f")
            tng = sb.tile([DIM, n], BF16, tag="lw_tg", bufs=1, name="tng")
            sig_o = sb.tile([DIM, n], BF16, tag="lw_so", bufs=1, name="sig_o")
            nc.scalar.activation(sig_i[:], gps[0][:], AF.Sigmoid, bias=bias[:, 0:1])
            nc.scalar.activation(sig_f[:], gps[1][:], AF.Sigmoid, bias=bias[:, 1:2])
            nc.scalar.activation(tng[:], gps[2][:], AF.Tanh, bias=bias[:, 2:3])
            nc.scalar.activation(sig_o[:], gps[3][:], AF.Sigmoid, bias=bias[:, 3:4])
            t1 = sb.tile([DIM, n], F32, tag="lw_t1", bufs=1, name="t1")
            nc.vector.tensor_mul(t1[:], sig_f[:], c_st[:, sl])
            t2 = sb.tile([DIM, n], F32, tag="lw_t2", bufs=1, name="t2")
            nc.vector.tensor_mul(t2[:], sig_i[:], tng[:])
            nc.vector.tensor_add(c_st[:, sl], t1[:], t2[:])
            tnc = sb.tile([DIM, n], BF16, tag="lw_tc", bufs=1, name="tnc")
            nc.scalar.activation(tnc[:], c_st[:, sl], AF.Tanh)
            nc.vector.tensor_mul(h_st[:, sl], sig_o[:], tnc[:])

        def c_phase(lct_ps):
            """C-LSTM + C_pre MLP + transposes -> cpre_kt [128, 16*128]."""
            lc_sb = sb.tile([DIM, CC], BF16, tag="lc_sb", bufs=1, name="lc_sb")
            for rc in range(4):
                sl = slice(rc * 512, (rc + 1) * 512)
                nc.vector.tensor_copy(lc_sb[:, sl], lct_ps[rc][:])
                gps = [ps.tile([DIM, 512], F32, tag="ps", name=f"cg{i}")
                       for i in range(4)]
                for g in range(4):
                    gsl = slice(g * DIM, (g + 1) * DIM)
                    nc.tensor.matmul(gps[g][:], cw["cu_wiht"][:, gsl],
                                     lc_sb[:, sl], start=True, stop=False)
                    nc.tensor.matmul(gps[g][:], cw["cu_whht"][:, gsl],
                                     cht[:, sl], start=False, stop=True)
                lstm_elementwise(gps, cw["cu_bias"], cct, cht, rc * 512, 512)
            cpreT = mlp_chunk(cht, "cmsg", slice(0, CC), CC)
            cpre_kt = sb.tile([DIM, KC * DIM], FP8, tag="cpre_kt", bufs=1,
                              name="cpre_kt")
            for t in range(KC):
                sl = slice(t * DIM, (t + 1) * DIM)
                pt = pstr.tile([DIM, DIM], BF16, tag="pstr", name="cp_tr")
                nc.tensor.transpose(pt[:], cpreT[:, sl], idt[:])
                nc.vector.tensor_copy(cpre_kt[:, sl], pt[:])
            return cpre_kt

        def gemm2_group(cpre_kt, g, rs_bufs, r):
            """One GEMM2 group: 4 psum accums over KC k-tiles; stage to RS buf."""
            cl_ps = [ps.tile([DIM, 512], F32, tag="ps", name=f"cl{g}_{i}")
                     for i in range(4)]
            for q in range(4):
                if g * 4 + q >= 11:
                    b2t = sb3.tile([DIM, 4 * 2048], FP8, tag="b2t", name="b2t")
                    nc.sync.dma_start(b2t[:], b2.ap()[g, q, :, :])
                    b2v = b2t[:].rearrange("p (t c) -> p t c", c=2048)
                else:
                    gsl = slice((g * 4 + q) * 8192, (g * 4 + q + 1) * 8192)
                    b2v = b2r[:, gsl].rearrange("p (t c) -> p t c", c=2048)
                for kk in (0, 2):
                    k = 4 * q + kk
                    ck = cpre_kt[:, k * DIM:(k + 2) * DIM].rearrange(
                        "p (j d) -> p j d", j=2)
                    for i in range(4):
                        nc.tensor.matmul(
                            cl_ps[i][:], ck,
                            b2v[:, kk:kk + 2, i * 512:(i + 1) * 512],
                            start=(k == 0), stop=(k == KC - 2),
                            perf_mode=mybir.MatmulPerfMode.DoubleRow)
            for i in range(4):
                j = J_SETS[g][i]
                h, blk = j % 2, j // 2
                cs = sb.tile([DIM, 512], FP8, tag="cl_st", name="cl_st")
                nc.vector.tensor_copy(cs[:], cl_ps[i][:])
                dma2(rs_bufs[h][blk * DIM:(blk + 1) * DIM, :], cs[:])

        def l_half(h, clt_h, lh_flip, r, n_warm, ag_in):
            """L-LSTM + L_pre MLP + transposes for local half h; returns ag_in."""
            sl = slice(h * 512, (h + 1) * 512)
            fsl = slice((1 - h) * 512, (2 - h) * 512)
            gps = [ps.tile([DIM, 512], F32, tag="ps", name=f"lg{h}_{i}")
                   for i in range(4)]
            for g in range(4):
                gsl = slice(g * DIM, (g + 1) * DIM)
                for wi in range(n_warm):
                    nc.tensor.matmul(gps[g][:], idt[:], zbf[:],
                                     start=(wi == 0), stop=False)
                nc.tensor.matmul(gps[g][:], cw["lu_wiht_cl"][:, gsl],
                                 clt_h[:], start=(n_warm == 0), stop=False)
                nc.tensor.matmul(gps[g][:], cw["lu_wiht_fl"][:, gsl],
                                 lh_flip[:, fsl], start=False, stop=False)
                nc.tensor.matmul(gps[g][:], cw["lu_whht"][:, gsl],
                                 lh_flip[:, sl], start=False, stop=True)
            lstm_elementwise(gps, cw["lu_bias"], lct, lht, h * 512, 512)
            lpre_h = mlp_chunk(lht, "lmsg", sl, 512, tagsfx=f"_{h}")
            for t in range(4):
                tsl = slice(t * DIM, (t + 1) * DIM)
                pt = pstr.tile([DIM, DIM], BF16, tag="pstr", name="lp_tr")
                nc.tensor.transpose(pt[:], lpre_h[:, tsl], idt[:])
                st = sb.tile([DIM, DIM], FP8, tag="tr_st", name="tr_st")
                nc.vector.tensor_copy(st[:], pt[:])
                nc.sync.dma_start(ag_in[tsl, :], st[:])

        def gemm1(lpre_sb, n_warm=0):
            """GEMM1: LC.T [128, 2048] psum accums over 64 packed k-tiles."""
            lct_ps = [ps.tile([DIM, 512], F32, tag="ps", name=f"g1_{i}")
                      for i in range(4)]
            for wi in range(n_warm):
                nc.tensor.matmul(lct_ps[wi % 4][:], idt[:], zbf[:],
                                 start=(wi < 4), stop=False)
            for grp in range(KL // 4):
                b1t = sb3.tile([DIM, 4 * CC], FP8, tag="b1t", name="b1t")
                nc.sync.dma_start(b1t[:], b1.ap()[grp, :, :])
                b1v = b1t[:].rearrange("p (t c) -> p t c", c=CC)
                for kk in (0, 2):
                    k = 4 * grp + kk
                    lf = lpre_sb[k // 8]
                    t0 = k % 8
                    lk = lf[:, t0 * DIM:(t0 + 2) * DIM].rearrange(
                        "p (j d) -> p j d", j=2)
                    for c4 in range(4):
                        nc.tensor.matmul(
                            lct_ps[c4][:], lk,
                            b1v[:, kk:kk + 2, c4 * 512:(c4 + 1) * 512],
                            start=(k == 0 and n_warm == 0),
                            stop=(k == KL - 2),
                            perf_mode=mybir.MatmulPerfMode.DoubleRow)
            return lct_ps

        def load_lpre(ag_outs):
            """Load AG halves as 8 SBUF chunks of 8 k-tiles each."""
            lpre_sb = []
            for c8 in range(8):
                lt = sb.tile([DIM, 8 * DIM], FP8, tag="lpf", bufs=6,
                             name=f"lpf{c8}")
                src = ag_outs[c8 // 4][(c8 % 4) * 1024:(c8 % 4 + 1) * 1024, :]
                s3 = src.rearrange("(t p) d -> p t d", p=DIM)
                d3 = lt[:].rearrange("p (t d) -> p t d", d=DIM)
                nc.sync.dma_start(d3[0:64], s3[0:64])
                nc.sync.dma_start(d3[64:DIM], s3[64:DIM])
                lpre_sb.append(lt)
            return lpre_sb

        rg = [list(range(N_CORES))]

        def collective(kind, op, cin, cout):
            nc.gpsimd.collective_compute(kind, op, replica_groups=rg,
                                         ins=[cin.opt()], outs=[cout.opt()])

        # ====== round 0 head: L_pre from Lh0 -> ag_in halves ======
        ag_ins = []
        for h in range(2):
            ag_in = dram.tile([512, DIM], FP8, tag=f"ag_in{h}",
                              name=f"ag_in{h}_init")
            lpre_h = mlp_chunk(lht, "lmsg", slice(h * 512, (h + 1) * 512),
                               512, tagsfx=f"_{h}")
            for t in range(4):
                tsl = slice(t * DIM, (t + 1) * DIM)
                pt = pstr.tile([DIM, DIM], BF16, tag="pstr", name="lp_tr0")
                nc.tensor.transpose(pt[:], lpre_h[:, tsl], idt[:])
                st = sb.tile([DIM, DIM], FP8, tag="tr_st", name="tr_st0")
                nc.vector.tensor_copy(st[:], pt[:])
                nc.sync.dma_start(ag_in[tsl, :], st[:])
            ag_ins.append(ag_in)

        for r in range(N_ROUNDS):
            ag_outs = []
            for h in range(2):
                ag_out = dram.tile([4096, DIM], FP8, tag=f"ag_out{h}",
                                   name=f"ag_out{h}_{r}")
                collective("AllGather", mybir.AluOpType.bypass,
                           ag_ins[h], ag_out)
                ag_outs.append(ag_out)
            lpre_sb = load_lpre(ag_outs)
            lct_ps = gemm1(lpre_sb, N_WARM_G1)
            cpre_kt = c_phase(lct_ps)

            rs_bufs = [dram.tile([N_CORES * DIM, 512], FP8, tag=f"rs_in{h}",
                                 name=f"rs_in{h}_{r}") for h in range(2)]
            gemm2_group(cpre_kt, 0, rs_bufs, r)
            gemm2_group(cpre_kt, 1, rs_bufs, r)
            ro0 = dram.tile([DIM, 512], FP8, tag="rs_out0", name=f"rs_out0_{r}")
            collective("ReduceScatter", mybir.AluOpType.add, rs_bufs[0], ro0)
            gemm2_group(cpre_kt, 2, rs_bufs, r)
            gemm2_group(cpre_kt, 3, rs_bufs, r)

            ro1 = dram.tile([DIM, 512], FP8, tag="rs_out1", name=f"rs_out1_{r}")
            collective("ReduceScatter", mybir.AluOpType.add, rs_bufs[1], ro1)

            lh_flip = sb.tile([DIM, LL], BF16, tag="lh_flip", bufs=1, name="lh_flip")
            nc.vector.tensor_copy(lh_flip[:], lht[:])

            ag_ins = [dram.tile([512, DIM], FP8, tag=f"ag_in{h}",
                                name=f"ag_in{h}_{r}") for h in range(2)]
            cl8_0 = sb.tile([DIM, 512], FP8, tag="cl8_0", name=f"cl8_0_{r}")
            dma2(cl8_0[:], ro0[:])
            clt0 = sb.tile([DIM, 512], BF16, tag="clt0", name=f"clt0_{r}")
            nc.vector.tensor_copy(clt0[:], cl8_0[:])
            l_half(0, clt0, lh_flip, r, N_WARM1, ag_ins[0])

            cl8_1 = sb.tile([DIM, 512], FP8, tag="cl8_1", name=f"cl8_1_{r}")
            dma2(cl8_1[:], ro1[:])
            clt1 = sb.tile([DIM, 512], BF16, tag="clt1", name=f"clt1_{r}")
            nc.vector.tensor_copy(clt1[:], cl8_1[:])
            l_half(1, clt1, lh_flip, r, N_WARM2, ag_ins[1])

        # ---- vote MLP on final Lh -> [1, 1024] f32
        vt0 = mlp_chunk(lht, "lvote", slice(0, 512), 512, out_dt=F32,
                        tagsfx="_0")
        vt1 = mlp_chunk(lht, "lvote", slice(512, 1024), 512, out_dt=F32,
                        tagsfx="_1")
        nc.sync.dma_start(vote_out.ap()[:, 0:512], vt0[:])
        nc.sync.dma_start(vote_out.ap()[:, 512:1024], vt1[:])

    nc.compile()
    _CACHE["nc"] = nc
    return nc


def _perm_rows(lits):
    """Map global lit index -> permuted row (core-major, 1024 rows/core)."""
    lits = np.asarray(lits)
    neg = lits >= N_VARS
    v = np.where(neg, lits - N_VARS, lits)
    core = v // VPC
    r = v % VPC
    return core * LL + np.where(neg, VPAD + r, r)


def _b1_row_order():
    """B1 rows: [half h, core c, r] -> permuted row c*1024 + h*512 + r."""
    order = np.empty(LPAD, np.int64)
    n = 0
    for h in range(2):
        for c in range(N_CORES):
            order[n:n + 512] = c * LL + h * 512 + np.arange(512)
            n += 512
    return order


def host_prep(inp):
    f32 = np.float32
    idx = inp["L_unpack_indices"].astype(np.int64)
    rows = _perm_rows(idx[:, 0])
    M = np.zeros((LPAD, CPAD), np.float32)
    np.add.at(M, (rows, idx[:, 1]), 1.0)

    row_order = _b1_row_order()
    b1s, b2s = [], []
    for i in range(N_CORES):
        blk = M[:, i * CC:(i + 1) * CC]          # [8192, 2048] permuted rows
        b1o = blk[row_order]                      # AG-concat row order
        # pack 4 k-tiles per DMA group: [16, 128, 4*2048]
        b1p = b1o.reshape(16, 4, DIM, CC).transpose(0, 2, 1, 3) \
                 .reshape(16, DIM, 4 * CC)
        b1s.append(np.ascontiguousarray(b1p).astype(nf8))
        bT = blk.T                                # [2048 clauses, 8192 lits]
        grp = []
        for g in range(4):
            cols = np.concatenate([np.arange(j * 512, (j + 1) * 512)
                                   for j in J_SETS[g]])
            gb = bT[:, cols]                      # [2048, 2048]
            gp = gb.reshape(4, 4, DIM, 2048).transpose(0, 2, 1, 3) \
                   .reshape(4, DIM, 4 * 2048)
            grp.append(gp)
        b2s.append(np.ascontiguousarray(np.stack(grp)).astype(nf8))

    def bf(x):
        return np.ascontiguousarray(x).astype(nbf)

    l0 = (inp["L_init_w"][:, 0] + inp["L_init_b"]).astype(f32)
    c0 = (inp["C_init_w"][:, 0] + inp["C_init_b"]).astype(f32)
    common = {
        "lh0t": bf(np.repeat(l0[:, None], LL, axis=1)),
        "ch0t": bf(np.repeat(c0[:, None], CC, axis=1)),
        "id128": bf(np.eye(DIM, dtype=f32)),
        "cu_wiht": bf(inp["Cu_wih"].T), "cu_whht": bf(inp["Cu_whh"].T),
        "lu_wiht_cl": bf(inp["Lu_wih"].T[:DIM]),
        "lu_wiht_fl": bf(inp["Lu_wih"].T[DIM:]),
        "lu_whht": bf(inp["Lu_whh"].T),
        "cu_bias": (inp["Cu_bih"] + inp["Cu_bhh"]).astype(f32).reshape(4, DIM),
        "lu_bias": (inp["Lu_bih"] + inp["Lu_bhh"]).astype(f32).reshape(4, DIM),
    }
    for p, P in (("lmsg", "Lmsg"), ("cmsg", "Cmsg"), ("lvote", "Lvote")):
        for i in (1, 2, 3):
            common[f"{p}_w{i}t"] = bf(inp[f"{P}_w{i}"].T)
            bshape = (1, 1) if (p == "lvote" and i == 3) else (DIM, 1)
            common[f"{p}_b{i}"] = inp[f"{P}_b{i}"].astype(f32).reshape(bshape)
    return [dict(common, b1=b1s[i], b2=b2s[i]) for i in range(N_CORES)]


def kernel(**inputs):
    inp = {k: np.asarray(v) for k, v in inputs.items()}
    in_maps = host_prep(inp)
    nc = _build()
    res = bass_utils.run_bass_kernel_spmd(nc, in_maps,
                                          core_ids=list(range(N_CORES)))
    probs = np.zeros(N_CORES, np.float32)
    for i in range(N_CORES):
        v = res.results[i]["vote"][0]            # [1024]
        s = v[:VPC].astype(np.float64).sum() + \
            v[VPAD:VPAD + VPC].astype(np.float64).sum()
        probs[i] = np.float32(s / (2 * VPC))
    return probs



# revision 12
# speedup vs baseline: 1.0860x; 1.0860x over previous
"""NeuroSAT message-passing GNN on 8 TRN2 NeuronCores (Bass/Tile).

Sharding: clause dim sharded 8-way (2048 padded clauses/core); literal dim
permuted so core i owns problem i's 500 vars (+12 pads) as 1024 lit rows
(512 pos + 512 neg).  Per round (pipelined):
  GEMM2 groups 0,1 -> AllToAll half0 ; groups 2,3 -> AllToAll half1 (fp8)
  partials summed locally on DVE (f32), L-LSTM + L_pre MLP per half,
  AllGather halves of L_pre (fp8, Shared-output Mesh)
  GEMM1 LC.T = L_pre.T @ B1 ; C-LSTM ; C_pre MLP ; repeat
M (counts) is exact in fp8e4m3; fp8 M blocks stream from HBM as the moving
operand against fp8 stationary activations (DoubleRow).  AllToAll is used
instead of ReduceScatter because it always runs the O(1)-hop Mesh algorithm
(RS picks RDH at this size: ~2x slower); the 8 partial blocks are reduced
on the vector engine.  Zero-contribution dummy matmuls keep the PE HAM-warm
(K=8/8 clock) across the residual collective waits.
"""

import numpy as np
import ml_dtypes

import concourse.bass as bass
import concourse.bacc as bacc
import concourse.mybir as mybir
import concourse.tile as tile
from concourse import bass_utils

F32 = mybir.dt.float32
BF16 = mybir.dt.bfloat16
FP8 = mybir.dt.float8e4
AF = mybir.ActivationFunctionType

N_CORES = 8
DIM = 128
N_ROUNDS = 16
N_VARS = 4000
VPC = 500            # real vars per core (= vars per problem)
VPAD = 512           # padded vars per core
LL = 2 * VPAD        # 1024 lit rows per core
LPAD = N_CORES * LL  # 8192
CC = 2048            # padded clauses per core
CPAD = N_CORES * CC  # 16384
KL = LPAD // 128     # 64 k-tiles over lits
KC = CC // 128       # 16 k-tiles over clauses

# GEMM2 groups: group g computes 512-lit chunks J_SETS[g]; chunk j covers
# local lit rows [512*(j%2)...) of destination core j//2.  Groups 0,1 cover
# all even j (RS half 0 = every core's rows 0:512); groups 2,3 odd j.
J_SETS = [[0, 2, 4, 6], [8, 10, 12, 14], [1, 3, 5, 7], [9, 11, 13, 15]]

N_WARM1 = 0          # dummy MMs per gate group, L half 0 (A2A_0 wait)
N_WARM_G1 = 16       # dummy MM prefix on GEMM1 (AG + load window)
N_WARM2 = 4          # dummy MMs per gate group, L half 1 (A2A_1 wait)
N_B2_RES = 8         # resident b2 blocks (of 16); rest streamed per round

nbf = ml_dtypes.bfloat16
nf8 = ml_dtypes.float8_e4m3

_CACHE = {}


def _build():
    """Build + compile the SPMD program once (shape-only, no input values)."""
    if "nc" in _CACHE:
        return _CACHE["nc"]

    nc = bacc.Bacc("TRN2", target_bir_lowering=False, debug=False,
                   num_devices=N_CORES)

    def din(name, shape, dt):
        return nc.dram_tensor(name, shape, dt, kind="ExternalInput")

    # b1: 16 packed groups of 4 k-tiles; rows ordered [half h, core c, r<512]
    b1 = din("b1", [KL // 4, DIM, 4 * CC], FP8)
    # b2[g]: group g's 16 k-tiles packed 4-per-DMA: [4 groups, 4 qgrp, 128, 4*2048]
    b2 = din("b2", [4, 4, DIM, 4 * 2048], FP8)
    lh0t = din("lh0t", [DIM, LL], BF16)
    ch0t = din("ch0t", [DIM, CC], BF16)
    id128 = din("id128", [DIM, DIM], BF16)

    w = {}
    for p in ("lmsg", "cmsg", "lvote"):
        for i in (1, 2, 3):
            shp = [DIM, 1] if (p == "lvote" and i == 3) else [DIM, DIM]
            w[f"{p}_w{i}t"] = din(f"{p}_w{i}t", shp, BF16)
            bshp = [1, 1] if (p == "lvote" and i == 3) else [DIM, 1]
            w[f"{p}_b{i}"] = din(f"{p}_b{i}", bshp, F32)
    w["cu_wiht"] = din("cu_wiht", [DIM, 4 * DIM], BF16)
    w["cu_whht"] = din("cu_whht", [DIM, 4 * DIM], BF16)
    w["lu_wiht_cl"] = din("lu_wiht_cl", [DIM, 4 * DIM], BF16)
    w["lu_wiht_fl"] = din("lu_wiht_fl", [DIM, 4 * DIM], BF16)
    w["lu_whht"] = din("lu_whht", [DIM, 4 * DIM], BF16)
    cu_bias_d = din("cu_bias", [4, DIM], F32)
    lu_bias_d = din("lu_bias", [4, DIM], F32)

    vote_out = nc.dram_tensor("vote", [1, LL], F32, kind="ExternalOutput")

    with tile.TileContext(nc) as tc, \
         tc.tile_pool(name="const", bufs=1) as const, \
         tc.tile_pool(name="sb", bufs=2) as sb, \
         tc.tile_pool(name="sb3", bufs=2) as sb3, \
         tc.tile_pool(name="ps", bufs=6, space="PSUM") as ps, \
         tc.tile_pool(name="pstr", bufs=2, space="PSUM") as pstr, \
         tc.tile_pool(name="dram", bufs=2, space="DRAM") as dram:

        # ---- resident b2: first N_B2_RES of 16 blocks of [128, 8192] fp8
        b2r = const.tile([DIM, N_B2_RES * 4 * 2048], FP8, tag="b2r")
        for g in range(4):
            for q in range(4):
                if g * 4 + q >= N_B2_RES:
                    continue
                sl = slice((g * 4 + q) * 8192, (g * 4 + q + 1) * 8192)
                nc.sync.dma_start(b2r[:, sl], b2.ap()[g, q, :, :])

        # ---- load constants/weights into SBUF
        cw = {}
        for k in w:
            t = const.tile(list(w[k].shape), w[k].dtype, tag=f"cw_{k}")
            nc.sync.dma_start(t[:], w[k].ap())
            cw[k] = t
        for k, dte in (("cu_bias", cu_bias_d), ("lu_bias", lu_bias_d)):
            t = const.tile([DIM, 4], F32, tag=f"cw_{k}")
            nc.sync.dma_start(t[:], dte.ap().rearrange("g p -> p g"))
            cw[k] = t
        idt = const.tile([DIM, DIM], BF16, tag="idt")
        nc.sync.dma_start(idt[:], id128.ap())
        zbf = const.tile([DIM, 512], BF16, tag="zbf")
        nc.vector.memset(zbf[:], 0.0)

        # ---- persistent state (feature-major)
        lht = const.tile([DIM, LL], BF16, tag="lht")
        lct = const.tile([DIM, LL], F32, tag="lct")
        cht = const.tile([DIM, CC], BF16, tag="cht")
        cct = const.tile([DIM, CC], F32, tag="cct")
        nc.sync.dma_start(lht[:], lh0t.ap())
        nc.sync.dma_start(cht[:], ch0t.ap())
        nc.vector.memset(lct[:], 0.0)
        nc.vector.memset(cct[:], 0.0)

        def dma2(dst, src):
            """Split a [128, N] transfer across two DMA queues by partitions."""
            nc.sync.dma_start(dst[0:64, :], src[0:64, :])
            nc.sync.dma_start(dst[64:DIM, :], src[64:DIM, :])

        def mlp_chunk(x, pfx, sl, n, out_dt=BF16, tagsfx=""):
            """3-layer MLP on columns sl (chunks of <=512) of x [128, *]."""
            cur = x
            for li in (1, 2, 3):
                wt = cw[f"{pfx}_w{li}t"]
                bt = cw[f"{pfx}_b{li}"]
                m = wt.shape[1]
                o = sb.tile([m, n], out_dt if li == 3 else BF16, bufs=1,
                            tag=f"{pfx}_h{li}{tagsfx}", name=f"{pfx}_h{li}{tagsfx}")
                for rc in range(n // 512):
                    c0 = rc * 512
                    pt = ps.tile([m, 512], F32, tag="ps", name="mlp_ps")
                    src = cur[:, sl.start + c0:sl.start + c0 + 512] if li == 1 \
                        else cur[:, c0:c0 + 512]
                    nc.tensor.matmul(pt[:], wt[:], src, start=True, stop=True)
                    func = AF.Relu if li < 3 else AF.Identity
                    nc.scalar.activation(o[:, c0:c0 + 512], pt[:], func,
                                         bias=bt[:, 0:1])
                cur = o
            return cur

        def lstm_elementwise(gps, bias, c_st, h_st, rc0, n):
            """gps: 4 psum tiles [128, n] (i,f,g,o); updates states [:, rc0:rc0+n]."""
            sl = slice(rc0, rc0 + n)
            sig_i = sb.tile([DIM, n], BF16, tag="lw_si", bufs=1, name="sig_i")
            sig_f = sb.tile([DIM, n], BF16, tag="lw_sf", bufs=1, name="sig_f")
            tng = sb.tile([DIM, n], BF16, tag="lw_tg", bufs=1, name="tng")
            sig_o = sb.tile([DIM, n], BF16, tag="lw_so", bufs=1, name="sig_o")
            nc.scalar.activation(sig_i[:], gps[0][:], AF.Sigmoid, bias=bias[:, 0:1])
            nc.scalar.activation(sig_f[:], gps[1][:], AF.Sigmoid, bias=bias[:, 1:2])
            nc.scalar.activation(tng[:], gps[2][:], AF.Tanh, bias=bias[:, 2:3])
            nc.scalar.activation(sig_o[:], gps[3][:], AF.Sigmoid, bias=bias[:, 3:4])
            t1 = sb.tile([DIM, n], F32, tag="lw_t1", bufs=1, name="t1")
            nc.vector.tensor_mul(t1[:], sig_f[:], c_st[:, sl])
            t2 = sb.tile([DIM, n], F32, tag="lw_t2", bufs=1, name="t2")
            nc.vector.tensor_mul(t2[:], sig_i[:], tng[:])
            nc.vector.tensor_add(c_st[:, sl], t1[:], t2[:])
            tnc = sb.tile([DIM, n], BF16, tag="lw_tc", bufs=1, name="tnc")
            nc.scalar.activation(tnc[:], c_st[:, sl], AF.Tanh)
            nc.vector.tensor_mul(h_st[:, sl], sig_o[:], tnc[:])

        def c_phase(lct_ps):
            """C-LSTM + C_pre MLP + transposes -> cpre_kt [128, 16*128]."""
            for rc in range(4):
                sl = slice(rc * 512, (rc + 1) * 512)
                lc_sb = sb.tile([DIM, 512], BF16, tag="lc_sb", bufs=2,
                                name=f"lc_sb{rc}")
                nc.vector.tensor_copy(lc_sb[:], lct_ps[rc][:])
                gps = [ps.tile([DIM, 512], F32, tag="ps", name=f"cg{i}")
                       for i in range(4)]
                for g in range(4):
                    gsl = slice(g * DIM, (g + 1) * DIM)
                    nc.tensor.matmul(gps[g][:], cw["cu_wiht"][:, gsl],
                                     lc_sb[:], start=True, stop=False)
                    nc.tensor.matmul(gps[g][:], cw["cu_whht"][:, gsl],
                                     cht[:, sl], start=False, stop=True)
                lstm_elementwise(gps, cw["cu_bias"], cct, cht, rc * 512, 512)
            cpre_kt = sb.tile([DIM, KC * DIM], FP8, tag="cpre_kt", bufs=1,
                              name="cpre_kt")
            for ch in range(2):
                cpreT = mlp_chunk(cht, "cmsg", slice(ch * 1024, (ch + 1) * 1024),
                                  1024)
                for t in range(8):
                    sl = slice(t * DIM, (t + 1) * DIM)
                    osl = slice((ch * 8 + t) * DIM, (ch * 8 + t + 1) * DIM)
                    pt = pstr.tile([DIM, DIM], BF16, tag="pstr", name="cp_tr")
                    nc.tensor.transpose(pt[:], cpreT[:, sl], idt[:])
                    nc.vector.tensor_copy(cpre_kt[:, osl], pt[:])
            return cpre_kt

        def gemm2_group(cpre_kt, g, rs_bufs, r, b2s):
            """One GEMM2 group: 4 psum accums over KC k-tiles; stage to RS buf."""
            cl_ps = [ps.tile([DIM, 512], F32, tag="ps", name=f"cl{g}_{i}")
                     for i in range(4)]
            for q in range(4):
                if g * 4 + q >= N_B2_RES:
                    b2t = b2s[g * 4 + q - N_B2_RES]
                    b2v = b2t[:].rearrange("p (t c) -> p t c", c=2048)
                else:
                    gsl = slice((g * 4 + q) * 8192, (g * 4 + q + 1) * 8192)
                    b2v = b2r[:, gsl].rearrange("p (t c) -> p t c", c=2048)
                for kk in (0, 2):
                    k = 4 * q + kk
                    ck = cpre_kt[:, k * DIM:(k + 2) * DIM].rearrange(
                        "p (j d) -> p j d", j=2)
                    for i in range(4):
                        nc.tensor.matmul(
                            cl_ps[i][:], ck,
                            b2v[:, kk:kk + 2, i * 512:(i + 1) * 512],
                            start=(k == 0), stop=(k == KC - 2),
                            perf_mode=mybir.MatmulPerfMode.DoubleRow)
            for i in range(4):
                j = J_SETS[g][i]
                h, blk = j % 2, j // 2
                cs = sb.tile([DIM, 512], FP8, tag="cl_st", name="cl_st")
                nc.vector.tensor_copy(cs[:], cl_ps[i][:])
                dma2(rs_bufs[h][blk * DIM:(blk + 1) * DIM, :], cs[:])

        def l_half(h, clt_h, lh_flip, r, n_warm, ag_in):
            """L-LSTM + L_pre MLP + transposes for local half h; returns ag_in."""
            sl = slice(h * 512, (h + 1) * 512)
            fsl = slice((1 - h) * 512, (2 - h) * 512)
            gps = [ps.tile([DIM, 512], F32, tag="ps", name=f"lg{h}_{i}")
                   for i in range(4)]
            for g in range(4):
                gsl = slice(g * DIM, (g + 1) * DIM)
                for wi in range(n_warm):
                    nc.tensor.matmul(gps[g][:], idt[:], zbf[:],
                                     start=(wi == 0), stop=False)
                nc.tensor.matmul(gps[g][:], cw["lu_wiht_cl"][:, gsl],
                                 clt_h[:], start=(n_warm == 0), stop=False)
                nc.tensor.matmul(gps[g][:], cw["lu_wiht_fl"][:, gsl],
                                 lh_flip[:, fsl], start=False, stop=False)
                nc.tensor.matmul(gps[g][:], cw["lu_whht"][:, gsl],
                                 lh_flip[:, sl], start=False, stop=True)
            lstm_elementwise(gps, cw["lu_bias"], lct, lht, h * 512, 512)
            lpre_h = mlp_chunk(lht, "lmsg", sl, 512, tagsfx=f"_{h}")
            for t in range(4):
                tsl = slice(t * DIM, (t + 1) * DIM)
                pt = pstr.tile([DIM, DIM], BF16, tag="pstr", name="lp_tr")
                nc.tensor.transpose(pt[:], lpre_h[:, tsl], idt[:])
                st = sb.tile([DIM, DIM], FP8, tag="tr_st", name="tr_st")
                nc.vector.tensor_copy(st[:], pt[:])
                nc.sync.dma_start(ag_in[tsl, :], st[:])

        def gemm1(lpre_sb, n_warm=0):
            """GEMM1: LC.T [128, 2048] psum accums over 64 packed k-tiles."""
            lct_ps = [ps.tile([DIM, 512], F32, tag="ps", name=f"g1_{i}")
                      for i in range(4)]
            for wi in range(n_warm):
                nc.tensor.matmul(lct_ps[wi % 4][:], idt[:], zbf[:],
                                 start=(wi < 4), stop=False)
            for grp in range(KL // 4):
                b1t = sb3.tile([DIM, 4 * CC], FP8, tag="b1t", bufs=4, name="b1t")
                nc.sync.dma_start(b1t[:], b1.ap()[grp, :, :])
                b1v = b1t[:].rearrange("p (t c) -> p t c", c=CC)
                for kk in (0, 2):
                    k = 4 * grp + kk
                    lf = lpre_sb[k // 8]
                    t0 = k % 8
                    lk = lf[:, t0 * DIM:(t0 + 2) * DIM].rearrange(
                        "p (j d) -> p j d", j=2)
                    for c4 in range(4):
                        nc.tensor.matmul(
                            lct_ps[c4][:], lk,
                            b1v[:, kk:kk + 2, c4 * 512:(c4 + 1) * 512],
                            start=(k == 0 and n_warm == 0),
                            stop=(k == KL - 2),
                            perf_mode=mybir.MatmulPerfMode.DoubleRow)
            return lct_ps

        def load_lpre(ag_outs):
            """Load AG halves as 8 SBUF chunks of 8 k-tiles each."""
            lpre_sb = []
            for c8 in range(8):
                lt = sb.tile([DIM, 8 * DIM], FP8, tag="lpf", bufs=6,
                             name=f"lpf{c8}")
                src = ag_outs[c8 // 4][(c8 % 4) * 1024:(c8 % 4 + 1) * 1024, :]
                s3 = src.rearrange("(t p) d -> p t d", p=DIM)
                d3 = lt[:].rearrange("p (t d) -> p t d", d=DIM)
                nc.sync.dma_start(d3[0:64], s3[0:64])
                nc.sync.dma_start(d3[64:DIM], s3[64:DIM])
                lpre_sb.append(lt)
            return lpre_sb

        rg = [list(range(N_CORES))]

        def collective(kind, op, cin, cout):
            nc.gpsimd.collective_compute(kind, op, replica_groups=rg,
                                         ins=[cin.opt()], outs=[cout.opt()])

        # ====== round 0 head: L_pre from Lh0 -> ag_in halves ======
        ag_ins = []
        for h in range(2):
            ag_in = dram.tile([512, DIM], FP8, tag=f"ag_in{h}",
                              name=f"ag_in{h}_init")
            lpre_h = mlp_chunk(lht, "lmsg", slice(h * 512, (h + 1) * 512),
                               512, tagsfx=f"_{h}")
            for t in range(4):
                tsl = slice(t * DIM, (t + 1) * DIM)
                pt = pstr.tile([DIM, DIM], BF16, tag="pstr", name="lp_tr0")
                nc.tensor.transpose(pt[:], lpre_h[:, tsl], idt[:])
                st = sb.tile([DIM, DIM], FP8, tag="tr_st", name="tr_st0")
                nc.vector.tensor_copy(st[:], pt[:])
                nc.sync.dma_start(ag_in[tsl, :], st[:])
            ag_ins.append(ag_in)

        def a2a_reduce(h, ro, r):
            """Load A2A output (8 partial blocks) and tree-sum -> clt bf16."""
            a2a_sb = sb.tile([DIM, 8 * 512], FP8, tag="a2a_sb", bufs=2,
                             name=f"a2a_sb{h}_{r}")
            src3 = ro[:].rearrange("(b p) c -> p b c", p=DIM)
            dst3 = a2a_sb[:].rearrange("p (b c) -> p b c", c=512)
            nc.sync.dma_start(dst3[0:64], src3[0:64])
            nc.sync.dma_start(dst3[64:DIM], src3[64:DIM])
            acc = sb.tile([DIM, 512], F32, tag="a2a_acc", bufs=1,
                          name=f"acc{h}_{r}")
            blk = lambda b: a2a_sb[:, b * 512:(b + 1) * 512]
            nc.vector.tensor_add(acc[:], blk(0), blk(1))
            for b in range(2, 7):
                nc.vector.tensor_add(acc[:], acc[:], blk(b))
            clt = sb.tile([DIM, 512], BF16, tag=f"clt{h}", bufs=1,
                          name=f"clt{h}_{r}")
            nc.vector.tensor_add(clt[:], acc[:], blk(7))
            return clt

        for r in range(N_ROUNDS):
            # prefetch the streamed b2 blocks for this round's GEMM2
            b2s = []
            for i in range(N_B2_RES, 16):
                b2t = sb3.tile([DIM, 4 * 2048], FP8, tag="b2t", bufs=3,
                               name=f"b2t_{i}")
                nc.sync.dma_start(b2t[:], b2.ap()[i // 4, i % 4, :, :])
                b2s.append(b2t)

            ag_outs = []
            for h in range(2):
                ag_out = dram.tile([4096, DIM], FP8, tag=f"ag_out{h}",
                                   addr_space="Shared", name=f"ag_out{h}_{r}")
                collective("AllGather", mybir.AluOpType.bypass,
                           ag_ins[h], ag_out)
                ag_outs.append(ag_out)
            lpre_sb = load_lpre(ag_outs)
            lct_ps = gemm1(lpre_sb, N_WARM_G1)
            cpre_kt = c_phase(lct_ps)

            rs_bufs = [dram.tile([N_CORES * DIM, 512], FP8, tag=f"rs_in{h}",
                                 name=f"rs_in{h}_{r}") for h in range(2)]
            gemm2_group(cpre_kt, 0, rs_bufs, r, b2s)
            gemm2_group(cpre_kt, 1, rs_bufs, r, b2s)
            ro0 = dram.tile([N_CORES * DIM, 512], FP8, tag="rs_out0",
                            name=f"rs_out0_{r}")
            collective("AllToAll", mybir.AluOpType.bypass, rs_bufs[0], ro0)
            gemm2_group(cpre_kt, 2, rs_bufs, r, b2s)
            gemm2_group(cpre_kt, 3, rs_bufs, r, b2s)

            ro1 = dram.tile([N_CORES * DIM, 512], FP8, tag="rs_out1",
                            name=f"rs_out1_{r}")
            collective("AllToAll", mybir.AluOpType.bypass, rs_bufs[1], ro1)

            lh_flip = sb.tile([DIM, LL], BF16, tag="lh_flip", bufs=1, name="lh_flip")
            nc.vector.tensor_copy(lh_flip[:], lht[:])

            ag_ins = [dram.tile([512, DIM], FP8, tag=f"ag_in{h}",
                                name=f"ag_in{h}_{r}") for h in range(2)]
            clt0 = a2a_reduce(0, ro0, r)
            l_half(0, clt0, lh_flip, r, N_WARM1, ag_ins[0])

            clt1 = a2a_reduce(1, ro1, r)
            l_half(1, clt1, lh_flip, r, N_WARM2, ag_ins[1])

        # ---- vote MLP on final Lh -> [1, 1024] f32
        vt0 = mlp_chunk(lht, "lvote", slice(0, 512), 512, out_dt=F32,
                        tagsfx="_0")
        vt1 = mlp_chunk(lht, "lvote", slice(512, 1024), 512, out_dt=F32,
                        tagsfx="_1")
        nc.sync.dma_start(vote_out.ap()[:, 0:512], vt0[:])
        nc.sync.dma_start(vote_out.ap()[:, 512:1024], vt1[:])

    nc.compile()
    _CACHE["nc"] = nc
    return nc


def _perm_rows(lits):
    """Map global lit index -> permuted row (core-major, 1024 rows/core)."""
    lits = np.asarray(lits)
    neg = lits >= N_VARS
    v = np.where(neg, lits - N_VARS, lits)
    core = v // VPC
    r = v % VPC
    return core * LL + np.where(neg, VPAD + r, r)


def _b1_row_order():
    """B1 rows: [half h, core c, r] -> permuted row c*1024 + h*512 + r."""
    order = np.empty(LPAD, np.int64)
    n = 0
    for h in range(2):
        for c in range(N_CORES):
            order[n:n + 512] = c * LL + h * 512 + np.arange(512)
            n += 512
    return order


def host_prep(inp):
    f32 = np.float32
    idx = inp["L_unpack_indices"].astype(np.int64)
    rows = _perm_rows(idx[:, 0])
    M = np.zeros((LPAD, CPAD), np.float32)
    np.add.at(M, (rows, idx[:, 1]), 1.0)

    row_order = _b1_row_order()
    b1s, b2s = [], []
    for i in range(N_CORES):
        blk = M[:, i * CC:(i + 1) * CC]          # [8192, 2048] permuted rows
        b1o = blk[row_order]                      # AG-concat row order
        # pack 4 k-tiles per DMA group: [16, 128, 4*2048]
        b1p = b1o.reshape(16, 4, DIM, CC).transpose(0, 2, 1, 3) \
                 .reshape(16, DIM, 4 * CC)
        b1s.append(np.ascontiguousarray(b1p).astype(nf8))
        bT = blk.T                                # [2048 clauses, 8192 lits]
        grp = []
        for g in range(4):
            cols = np.concatenate([np.arange(j * 512, (j + 1) * 512)
                                   for j in J_SETS[g]])
            gb = bT[:, cols]                      # [2048, 2048]
            gp = gb.reshape(4, 4, DIM, 2048).transpose(0, 2, 1, 3) \
                   .reshape(4, DIM, 4 * 2048)
            grp.append(gp)
        b2s.append(np.ascontiguousarray(np.stack(grp)).astype(nf8))

    def bf(x):
        return np.ascontiguousarray(x).astype(nbf)

    l0 = (inp["L_init_w"][:, 0] + inp["L_init_b"]).astype(f32)
    c0 = (inp["C_init_w"][:, 0] + inp["C_init_b"]).astype(f32)
    common = {
        "lh0t": bf(np.repeat(l0[:, None], LL, axis=1)),
        "ch0t": bf(np.repeat(c0[:, None], CC, axis=1)),
        "id128": bf(np.eye(DIM, dtype=f32)),
        "cu_wiht": bf(inp["Cu_wih"].T), "cu_whht": bf(inp["Cu_whh"].T),
        "lu_wiht_cl": bf(inp["Lu_wih"].T[:DIM]),
        "lu_wiht_fl": bf(inp["Lu_wih"].T[DIM:]),
        "lu_whht": bf(inp["Lu_whh"].T),
        "cu_bias": (inp["Cu_bih"] + inp["Cu_bhh"]).astype(f32).reshape(4, DIM),
        "lu_bias": (inp["Lu_bih"] + inp["Lu_bhh"]).astype(f32).reshape(4, DIM),
    }
    for p, P in (("lmsg", "Lmsg"), ("cmsg", "Cmsg"), ("lvote", "Lvote")):
        for i in (1, 2, 3):
            common[f"{p}_w{i}t"] = bf(inp[f"{P}_w{i}"].T)
            bshape = (1, 1) if (p == "lvote" and i == 3) else (DIM, 1)
            common[f"{p}_b{i}"] = inp[f"{P}_b{i}"].astype(f32).reshape(bshape)
    return [dict(common, b1=b1s[i], b2=b2s[i]) for i in range(N_CORES)]


def kernel(**inputs):
    inp = {k: np.asarray(v) for k, v in inputs.items()}
    in_maps = host_prep(inp)
    nc = _build()
    res = bass_utils.run_bass_kernel_spmd(nc, in_maps,
                                          core_ids=list(range(N_CORES)))
    probs = np.zeros(N_CORES, np.float32)
    for i in range(N_CORES):
        v = res.results[i]["vote"][0]            # [1024]
        s = v[:VPC].astype(np.float64).sum() + \
            v[VPAD:VPAD + VPC].astype(np.float64).sum()
        probs[i] = np.float32(s / (2 * VPC))
    return probs



# revision 19
# speedup vs baseline: 1.1576x; 1.0659x over previous
"""NeuroSAT message-passing GNN on 8 TRN2 NeuronCores (Bass/Tile).

Sharding: clause dim sharded 8-way (2048 padded clauses/core); literal dim
permuted so core i owns problem i's 500 vars (+12 pads) as 1024 lit rows
(512 pos + 512 neg).  Per round (pipelined):
  GEMM2 groups 0,1 -> AllToAll half0 ; groups 2,3 -> AllToAll half1 (fp8)
  partials summed locally on DVE (f32), L-LSTM + L_pre MLP per half,
  AllGather halves of L_pre (fp8, Shared-output Mesh)
  GEMM1 LC.T = L_pre.T @ B1 ; C-LSTM ; C_pre MLP ; repeat
M (counts) is exact in fp8e4m3; fp8 M blocks stream from HBM as the moving
operand against fp8 stationary activations (DoubleRow).  AllToAll is used
instead of ReduceScatter because it always runs the O(1)-hop Mesh algorithm
(RS picks RDH at this size: ~2x slower); the 8 partial blocks are reduced
on the vector engine.  Zero-contribution dummy matmuls keep the PE HAM-warm
(K=8/8 clock) across the residual collective waits.
"""

import numpy as np
import ml_dtypes

import concourse.bass as bass
import concourse.bacc as bacc
import concourse.mybir as mybir
import concourse.tile as tile
from concourse import bass_utils

F32 = mybir.dt.float32
BF16 = mybir.dt.bfloat16
FP8 = mybir.dt.float8e4
AF = mybir.ActivationFunctionType

N_CORES = 8
DIM = 128
N_ROUNDS = 16
N_VARS = 4000
VPC = 500            # real vars per core (= vars per problem)
VPAD = 512           # padded vars per core
LL = 2 * VPAD        # 1024 lit rows per core
LPAD = N_CORES * LL  # 8192
CC = 2048            # padded clauses per core
CPAD = N_CORES * CC  # 16384
KL = LPAD // 128     # 64 k-tiles over lits
KC = CC // 128       # 16 k-tiles over clauses

# GEMM2 groups: group g computes 512-lit chunks J_SETS[g]; chunk j covers
# local lit rows [512*(j%2)...) of destination core j//2.  Groups 0,1 cover
# all even j (RS half 0 = every core's rows 0:512); groups 2,3 odd j.
J_SETS = [[0, 2, 4, 6], [8, 10, 12, 14], [1, 3, 5, 7], [9, 11, 13, 15]]

N_WARM1 = 0          # dummy MMs per gate group, L half 0 (A2A_0 wait)
N_WARM_G1 = 8        # dummy MM prefix on GEMM1 (AG + load window)
N_WARM2 = 0          # dummy MMs per gate group, L half 1 (A2A_1 wait)
N_B2_RES = 8         # resident b2 blocks (of 16); rest streamed per round

nbf = ml_dtypes.bfloat16
nf8 = ml_dtypes.float8_e4m3

_CACHE = {}


def _build():
    """Build + compile the SPMD program once (shape-only, no input values)."""
    if "nc" in _CACHE:
        return _CACHE["nc"]

    nc = bacc.Bacc("TRN2", target_bir_lowering=False, debug=False,
                   num_devices=N_CORES)

    def din(name, shape, dt):
        return nc.dram_tensor(name, shape, dt, kind="ExternalInput")

    # b1: 16 packed groups of 4 k-tiles; rows ordered [half h, core c, r<512]
    b1 = din("b1", [KL // 4, DIM, 4 * CC], FP8)
    # b2[g]: group g's 16 k-tiles packed 4-per-DMA: [4 groups, 4 qgrp, 128, 4*2048]
    b2 = din("b2", [4, 4, DIM, 4 * 2048], FP8)
    lh0t = din("lh0t", [DIM, LL], BF16)
    ch0t = din("ch0t", [DIM, CC], BF16)
    id128 = din("id128", [DIM, DIM], BF16)

    w = {}
    for p in ("lmsg", "cmsg", "lvote"):
        for i in (1, 2, 3):
            shp = [DIM, 1] if (p == "lvote" and i == 3) else [DIM, DIM]
            w[f"{p}_w{i}t"] = din(f"{p}_w{i}t", shp, BF16)
            bshp = [1, 1] if (p == "lvote" and i == 3) else [DIM, 1]
            w[f"{p}_b{i}"] = din(f"{p}_b{i}", bshp, F32)
    w["cu_wiht"] = din("cu_wiht", [DIM, 4 * DIM], BF16)
    w["cu_whht"] = din("cu_whht", [DIM, 4 * DIM], BF16)
    w["lu_wiht_cl"] = din("lu_wiht_cl", [DIM, 4 * DIM], BF16)
    w["lu_wiht_fl"] = din("lu_wiht_fl", [DIM, 4 * DIM], BF16)
    w["lu_whht"] = din("lu_whht", [DIM, 4 * DIM], BF16)
    cu_bias_d = din("cu_bias", [4, DIM], F32)
    lu_bias_d = din("lu_bias", [4, DIM], F32)

    vote_out = nc.dram_tensor("vote", [1, LL], F32, kind="ExternalOutput")

    with tile.TileContext(nc) as tc, \
         tc.tile_pool(name="const", bufs=1) as const, \
         tc.tile_pool(name="sb", bufs=2) as sb, \
         tc.tile_pool(name="sb3", bufs=2) as sb3, \
         tc.tile_pool(name="ps", bufs=6, space="PSUM") as ps, \
         tc.tile_pool(name="pstr", bufs=2, space="PSUM") as pstr, \
         tc.tile_pool(name="dram", bufs=2, space="DRAM") as dram:

        # ---- resident b2: first N_B2_RES of 16 blocks of [128, 8192] fp8
        b2r = const.tile([DIM, N_B2_RES * 4 * 2048], FP8, tag="b2r")
        for g in range(4):
            for q in range(4):
                if g * 4 + q >= N_B2_RES:
                    continue
                sl = slice((g * 4 + q) * 8192, (g * 4 + q + 1) * 8192)
                nc.sync.dma_start(b2r[:, sl], b2.ap()[g, q, :, :])

        # ---- load constants/weights into SBUF
        cw = {}
        for k in w:
            t = const.tile(list(w[k].shape), w[k].dtype, tag=f"cw_{k}")
            nc.sync.dma_start(t[:], w[k].ap())
            cw[k] = t
        for k, dte in (("cu_bias", cu_bias_d), ("lu_bias", lu_bias_d)):
            t = const.tile([DIM, 4], F32, tag=f"cw_{k}")
            nc.sync.dma_start(t[:], dte.ap().rearrange("g p -> p g"))
            cw[k] = t
        idt = const.tile([DIM, DIM], BF16, tag="idt")
        nc.sync.dma_start(idt[:], id128.ap())
        zbf = const.tile([DIM, 512], BF16, tag="zbf")
        nc.vector.memset(zbf[:], 0.0)

        # ---- persistent state (feature-major)
        lht = const.tile([DIM, LL], BF16, tag="lht")
        lct = const.tile([DIM, LL], F32, tag="lct")
        cht = const.tile([DIM, CC], BF16, tag="cht")
        cct = const.tile([DIM, CC], F32, tag="cct")
        nc.sync.dma_start(lht[:], lh0t.ap())
        nc.sync.dma_start(cht[:], ch0t.ap())
        nc.vector.memset(lct[:], 0.0)
        nc.vector.memset(cct[:], 0.0)

        def dma2(dst, src):
            """Split a [128, N] transfer across two DMA queues by partitions."""
            nc.sync.dma_start(dst[0:64, :], src[0:64, :])
            nc.sync.dma_start(dst[64:DIM, :], src[64:DIM, :])

        def mlp_chunk(x, pfx, sl, n, out_dt=BF16, tagsfx=""):
            """3-layer MLP on columns sl (chunks of <=512) of x [128, *]."""
            cur = x
            for li in (1, 2, 3):
                wt = cw[f"{pfx}_w{li}t"]
                bt = cw[f"{pfx}_b{li}"]
                m = wt.shape[1]
                o = sb.tile([m, n], out_dt if li == 3 else BF16, bufs=1,
                            tag=f"{pfx}_h{li}{tagsfx}", name=f"{pfx}_h{li}{tagsfx}")
                for rc in range(n // 512):
                    c0 = rc * 512
                    pt = ps.tile([m, 512], F32, tag="ps", name="mlp_ps")
                    src = cur[:, sl.start + c0:sl.start + c0 + 512] if li == 1 \
                        else cur[:, c0:c0 + 512]
                    nc.tensor.matmul(pt[:], wt[:], src, start=True, stop=True)
                    func = AF.Relu if li < 3 else AF.Identity
                    nc.scalar.activation(o[:, c0:c0 + 512], pt[:], func,
                                         bias=bt[:, 0:1])
                cur = o
            return cur

        def lstm_elementwise(gps, bias, c_st, h_st, rc0, n):
            """gps: 4 psum tiles [128, n] (i,f,g,o); updates states [:, rc0:rc0+n]."""
            sl = slice(rc0, rc0 + n)
            sig_i = sb.tile([DIM, n], BF16, tag="lw_si", bufs=1, name="sig_i")
            sig_f = sb.tile([DIM, n], BF16, tag="lw_sf", bufs=1, name="sig_f")
            tng = sb.tile([DIM, n], BF16, tag="lw_tg", bufs=1, name="tng")
            sig_o = sb.tile([DIM, n], BF16, tag="lw_so", bufs=1, name="sig_o")
            nc.scalar.activation(sig_i[:], gps[0][:], AF.Sigmoid, bias=bias[:, 0:1])
            nc.scalar.activation(sig_f[:], gps[1][:], AF.Sigmoid, bias=bias[:, 1:2])
            nc.scalar.activation(tng[:], gps[2][:], AF.Tanh, bias=bias[:, 2:3])
            nc.scalar.activation(sig_o[:], gps[3][:], AF.Sigmoid, bias=bias[:, 3:4])
            t1 = sb.tile([DIM, n], F32, tag="lw_t1", bufs=1, name="t1")
            nc.vector.tensor_mul(t1[:], sig_f[:], c_st[:, sl])
            t2 = sb.tile([DIM, n], F32, tag="lw_t2", bufs=1, name="t2")
            nc.vector.tensor_mul(t2[:], sig_i[:], tng[:])
            nc.vector.tensor_add(c_st[:, sl], t1[:], t2[:])
            tnc = sb.tile([DIM, n], BF16, tag="lw_tc", bufs=1, name="tnc")
            nc.scalar.activation(tnc[:], c_st[:, sl], AF.Tanh)
            nc.vector.tensor_mul(h_st[:, sl], sig_o[:], tnc[:])

        def c_phase(lct_ps):
            """C-LSTM + C_pre MLP + transposes -> cpre_kt [128, 16*128]."""
            for rc in range(4):
                sl = slice(rc * 512, (rc + 1) * 512)
                lc_sb = sb.tile([DIM, 512], BF16, tag="lc_sb", bufs=2,
                                name=f"lc_sb{rc}")
                nc.vector.tensor_copy(lc_sb[:], lct_ps[rc][:])
                gps = [ps.tile([DIM, 512], F32, tag="ps", name=f"cg{i}")
                       for i in range(4)]
                for g in range(4):
                    gsl = slice(g * DIM, (g + 1) * DIM)
                    nc.tensor.matmul(gps[g][:], cw["cu_wiht"][:, gsl],
                                     lc_sb[:], start=True, stop=False)
                    nc.tensor.matmul(gps[g][:], cw["cu_whht"][:, gsl],
                                     cht[:, sl], start=False, stop=True)
                lstm_elementwise(gps, cw["cu_bias"], cct, cht, rc * 512, 512)
            cpre_kt = sb.tile([DIM, KC * DIM], FP8, tag="cpre_kt", bufs=1,
                              name="cpre_kt")
            for ch in range(2):
                cpreT = mlp_chunk(cht, "cmsg", slice(ch * 1024, (ch + 1) * 1024),
                                  1024)
                for t in range(8):
                    sl = slice(t * DIM, (t + 1) * DIM)
                    osl = slice((ch * 8 + t) * DIM, (ch * 8 + t + 1) * DIM)
                    pt = pstr.tile([DIM, DIM], BF16, tag="pstr", name="cp_tr")
                    nc.tensor.transpose(pt[:], cpreT[:, sl], idt[:])
                    nc.vector.tensor_copy(cpre_kt[:, osl], pt[:])
            return cpre_kt

        def gemm2_group(cpre_kt, g, rs_bufs, r, b2s):
            """One GEMM2 group: 4 psum accums over KC k-tiles; stage to RS buf."""
            cl_ps = [ps.tile([DIM, 512], F32, tag="ps", name=f"cl{g}_{i}")
                     for i in range(4)]
            for q in range(4):
                if g * 4 + q >= N_B2_RES:
                    b2t = b2s[g * 4 + q - N_B2_RES]
                    b2v = b2t[:].rearrange("p (t c) -> p t c", c=2048)
                else:
                    gsl = slice((g * 4 + q) * 8192, (g * 4 + q + 1) * 8192)
                    b2v = b2r[:, gsl].rearrange("p (t c) -> p t c", c=2048)
                for kk in (0, 2):
                    k = 4 * q + kk
                    ck = cpre_kt[:, k * DIM:(k + 2) * DIM].rearrange(
                        "p (j d) -> p j d", j=2)
                    for i in range(4):
                        nc.tensor.matmul(
                            cl_ps[i][:], ck,
                            b2v[:, kk:kk + 2, i * 512:(i + 1) * 512],
                            start=(k == 0), stop=(k == KC - 2),
                            perf_mode=mybir.MatmulPerfMode.DoubleRow)
            for i in range(4):
                j = J_SETS[g][i]
                h, blk = j % 2, j // 2
                cs = sb.tile([DIM, 512], FP8, tag="cl_st", name="cl_st")
                nc.vector.tensor_copy(cs[:], cl_ps[i][:])
                dma2(rs_bufs[h][blk * DIM:(blk + 1) * DIM, :], cs[:])

        def l_half(h, clt_h, lh_flip, r, n_warm, ag_in):
            """L-LSTM + L_pre MLP + transposes for local half h; returns ag_in."""
            sl = slice(h * 512, (h + 1) * 512)
            fsl = slice((1 - h) * 512, (2 - h) * 512)
            gps = [ps.tile([DIM, 512], F32, tag="ps", name=f"lg{h}_{i}")
                   for i in range(4)]
            # flip/hidden gate matmuls first: they only need lh_flip, so the
            # PE computes them while the A2A exchange is still in flight; the
            # clt matmuls (which wait on the reduce) come last.
            for g in range(4):
                gsl = slice(g * DIM, (g + 1) * DIM)
                for wi in range(n_warm):
                    nc.tensor.matmul(gps[g][:], idt[:], zbf[:],
                                     start=(wi == 0), stop=False)
                nc.tensor.matmul(gps[g][:], cw["lu_wiht_fl"][:, gsl],
                                 lh_flip[:, fsl], start=(n_warm == 0),
                                 stop=False)
                nc.tensor.matmul(gps[g][:], cw["lu_whht"][:, gsl],
                                 lh_flip[:, sl], start=False, stop=False)
            for g in range(4):
                gsl = slice(g * DIM, (g + 1) * DIM)
                nc.tensor.matmul(gps[g][:], cw["lu_wiht_cl"][:, gsl],
                                 clt_h[:], start=False, stop=True)
            lstm_elementwise(gps, cw["lu_bias"], lct, lht, h * 512, 512)
            lpre_h = mlp_chunk(lht, "lmsg", sl, 512, tagsfx=f"_{h}")
            for t in range(4):
                tsl = slice(t * DIM, (t + 1) * DIM)
                pt = pstr.tile([DIM, DIM], BF16, tag="pstr", name="lp_tr")
                nc.tensor.transpose(pt[:], lpre_h[:, tsl], idt[:])
                st = sb.tile([DIM, DIM], FP8, tag="tr_st", name="tr_st")
                nc.vector.tensor_copy(st[:], pt[:])
                nc.sync.dma_start(ag_in[tsl, :], st[:])

        def gemm1(lpre_sb, n_warm=0):
            """GEMM1: LC.T [128, 2048] psum accums over 64 packed k-tiles."""
            lct_ps = [ps.tile([DIM, 512], F32, tag="ps", name=f"g1_{i}")
                      for i in range(4)]
            for wi in range(n_warm):
                nc.tensor.matmul(lct_ps[wi % 4][:], idt[:], zbf[:],
                                 start=(wi < 4), stop=False)
            for grp in range(KL // 4):
                b1t = sb3.tile([DIM, 4 * CC], FP8, tag="b1t", bufs=4, name="b1t")
                nc.scalar.dma_start(b1t[:], b1.ap()[grp, :, :])
                b1v = b1t[:].rearrange("p (t c) -> p t c", c=CC)
                for kk in (0, 2):
                    k = 4 * grp + kk
                    lf = lpre_sb[k // 8]
                    t0 = k % 8
                    lk = lf[:, t0 * DIM:(t0 + 2) * DIM].rearrange(
                        "p (j d) -> p j d", j=2)
                    for c4 in range(4):
                        nc.tensor.matmul(
                            lct_ps[c4][:], lk,
                            b1v[:, kk:kk + 2, c4 * 512:(c4 + 1) * 512],
                            start=(k == 0 and n_warm == 0),
                            stop=(k == KL - 2),
                            perf_mode=mybir.MatmulPerfMode.DoubleRow)
            return lct_ps

        def load_lpre(ag_outs):
            """Load AG halves as 8 SBUF chunks of 8 k-tiles each."""
            lpre_sb = []
            for c8 in range(8):
                lt = sb.tile([DIM, 8 * DIM], FP8, tag="lpf", bufs=6,
                             name=f"lpf{c8}")
                src = ag_outs[c8 // 4][(c8 % 4) * 1024:(c8 % 4 + 1) * 1024, :]
                s3 = src.rearrange("(t p) d -> p t d", p=DIM)
                d3 = lt[:].rearrange("p (t d) -> p t d", d=DIM)
                nc.sync.dma_start(d3[0:64], s3[0:64])
                nc.sync.dma_start(d3[64:DIM], s3[64:DIM])
                lpre_sb.append(lt)
            return lpre_sb

        rg = [list(range(N_CORES))]

        def collective(kind, op, cin, cout):
            nc.gpsimd.collective_compute(kind, op, replica_groups=rg,
                                         ins=[cin.opt()], outs=[cout.opt()])

        # ====== round 0 head: L_pre from Lh0 -> ag_in halves ======
        ag_ins = []
        for h in range(2):
            ag_in = dram.tile([512, DIM], FP8, tag=f"ag_in{h}",
                              name=f"ag_in{h}_init")
            lpre_h = mlp_chunk(lht, "lmsg", slice(h * 512, (h + 1) * 512),
                               512, tagsfx=f"_{h}")
            for t in range(4):
                tsl = slice(t * DIM, (t + 1) * DIM)
                pt = pstr.tile([DIM, DIM], BF16, tag="pstr", name="lp_tr0")
                nc.tensor.transpose(pt[:], lpre_h[:, tsl], idt[:])
                st = sb.tile([DIM, DIM], FP8, tag="tr_st", name="tr_st0")
                nc.vector.tensor_copy(st[:], pt[:])
                nc.sync.dma_start(ag_in[tsl, :], st[:])
            ag_ins.append(ag_in)

        def a2a_reduce(h, ro, r):
            """Load A2A output (8 partial blocks) and tree-sum -> clt bf16.

            Stage-1 sums of two fp8 values are exact in bf16; the 3-level
            tree keeps the DVE ops independent (no serial RAW chain)."""
            a2a_sb = sb.tile([DIM, 8 * 512], FP8, tag="a2a_sb", bufs=2,
                             name=f"a2a_sb{h}_{r}")
            src3 = ro[:].rearrange("(b p) c -> p b c", p=DIM)
            dst3 = a2a_sb[:].rearrange("p (b c) -> p b c", c=512)
            nc.sync.dma_start(dst3[0:64], src3[0:64])
            nc.sync.dma_start(dst3[64:DIM], src3[64:DIM])
            blk = lambda b: a2a_sb[:, b * 512:(b + 1) * 512]
            s1 = sb.tile([DIM, 4 * 512], BF16, tag="a2a_s1", bufs=1,
                         name=f"s1_{h}_{r}")
            for b in range(4):
                nc.vector.tensor_add(s1[:, b * 512:(b + 1) * 512],
                                     blk(2 * b), blk(2 * b + 1))
            s2 = sb.tile([DIM, 2 * 512], F32, tag="a2a_s2", bufs=1,
                         name=f"s2_{h}_{r}")
            for b in range(2):
                nc.vector.tensor_add(s2[:, b * 512:(b + 1) * 512],
                                     s1[:, 2 * b * 512:(2 * b + 1) * 512],
                                     s1[:, (2 * b + 1) * 512:(2 * b + 2) * 512])
            clt = sb.tile([DIM, 512], BF16, tag=f"clt{h}", bufs=1,
                          name=f"clt{h}_{r}")
            nc.vector.tensor_add(clt[:], s2[:, 0:512], s2[:, 512:1024])
            return clt

        for r in range(N_ROUNDS):
            ag_outs = []
            for h in range(2):
                ag_out = dram.tile([4096, DIM], FP8, tag=f"ag_out{h}",
                                   addr_space="Shared", name=f"ag_out{h}_{r}")
                collective("AllGather", mybir.AluOpType.bypass,
                           ag_ins[h], ag_out)
                ag_outs.append(ag_out)
            lpre_sb = load_lpre(ag_outs)
            lct_ps = gemm1(lpre_sb, N_WARM_G1)

            # prefetch this round's streamed b2 blocks on the scalar HWDGE
            # ring, queued behind the b1 stream so they fill during c_phase
            b2s = []
            for i in range(N_B2_RES, 16):
                b2t = sb3.tile([DIM, 4 * 2048], FP8, tag="b2t", bufs=3,
                               name=f"b2t_{i}")
                nc.scalar.dma_start(b2t[:], b2.ap()[i // 4, i % 4, :, :])
                b2s.append(b2t)

            cpre_kt = c_phase(lct_ps)

            lh_flip = sb.tile([DIM, LL], BF16, tag="lh_flip", bufs=1,
                              name="lh_flip")
            nc.vector.tensor_copy(lh_flip[:], lht[:])

            rs_bufs = [dram.tile([N_CORES * DIM, 512], FP8, tag=f"rs_in{h}",
                                 name=f"rs_in{h}_{r}") for h in range(2)]
            gemm2_group(cpre_kt, 0, rs_bufs, r, b2s)
            gemm2_group(cpre_kt, 1, rs_bufs, r, b2s)
            ro0 = dram.tile([N_CORES * DIM, 512], FP8, tag="rs_out0",
                            name=f"rs_out0_{r}")
            collective("AllToAll", mybir.AluOpType.bypass, rs_bufs[0], ro0)
            gemm2_group(cpre_kt, 2, rs_bufs, r, b2s)
            gemm2_group(cpre_kt, 3, rs_bufs, r, b2s)

            ro1 = dram.tile([N_CORES * DIM, 512], FP8, tag="rs_out1",
                            name=f"rs_out1_{r}")
            collective("AllToAll", mybir.AluOpType.bypass, rs_bufs[1], ro1)

            ag_ins = [dram.tile([512, DIM], FP8, tag=f"ag_in{h}",
                                name=f"ag_in{h}_{r}") for h in range(2)]
            clt0 = a2a_reduce(0, ro0, r)
            l_half(0, clt0, lh_flip, r, N_WARM1, ag_ins[0])

            clt1 = a2a_reduce(1, ro1, r)
            l_half(1, clt1, lh_flip, r, N_WARM2, ag_ins[1])

        # ---- vote MLP on final Lh -> [1, 1024] f32
        vt0 = mlp_chunk(lht, "lvote", slice(0, 512), 512, out_dt=F32,
                        tagsfx="_0")
        vt1 = mlp_chunk(lht, "lvote", slice(512, 1024), 512, out_dt=F32,
                        tagsfx="_1")
        nc.sync.dma_start(vote_out.ap()[:, 0:512], vt0[:])
        nc.sync.dma_start(vote_out.ap()[:, 512:1024], vt1[:])

    nc.compile()
    _CACHE["nc"] = nc
    return nc


def _perm_rows(lits):
    """Map global lit index -> permuted row (core-major, 1024 rows/core)."""
    lits = np.asarray(lits)
    neg = lits >= N_VARS
    v = np.where(neg, lits - N_VARS, lits)
    core = v // VPC
    r = v % VPC
    return core * LL + np.where(neg, VPAD + r, r)


def _b1_row_order():
    """B1 rows: [half h, core c, r] -> permuted row c*1024 + h*512 + r."""
    order = np.empty(LPAD, np.int64)
    n = 0
    for h in range(2):
        for c in range(N_CORES):
            order[n:n + 512] = c * LL + h * 512 + np.arange(512)
            n += 512
    return order


def host_prep(inp):
    f32 = np.float32
    idx = inp["L_unpack_indices"].astype(np.int64)
    rows = _perm_rows(idx[:, 0])
    M = np.zeros((LPAD, CPAD), np.float32)
    np.add.at(M, (rows, idx[:, 1]), 1.0)

    row_order = _b1_row_order()
    b1s, b2s = [], []
    for i in range(N_CORES):
        blk = M[:, i * CC:(i + 1) * CC]          # [8192, 2048] permuted rows
        b1o = blk[row_order]                      # AG-concat row order
        # pack 4 k-tiles per DMA group: [16, 128, 4*2048]
        b1p = b1o.reshape(16, 4, DIM, CC).transpose(0, 2, 1, 3) \
                 .reshape(16, DIM, 4 * CC)
        b1s.append(np.ascontiguousarray(b1p).astype(nf8))
        bT = blk.T                                # [2048 clauses, 8192 lits]
        grp = []
        for g in range(4):
            cols = np.concatenate([np.arange(j * 512, (j + 1) * 512)
                                   for j in J_SETS[g]])
            gb = bT[:, cols]                      # [2048, 2048]
            gp = gb.reshape(4, 4, DIM, 2048).transpose(0, 2, 1, 3) \
                   .reshape(4, DIM, 4 * 2048)
            grp.append(gp)
        b2s.append(np.ascontiguousarray(np.stack(grp)).astype(nf8))

    def bf(x):
        return np.ascontiguousarray(x).astype(nbf)

    l0 = (inp["L_init_w"][:, 0] + inp["L_init_b"]).astype(f32)
    c0 = (inp["C_init_w"][:, 0] + inp["C_init_b"]).astype(f32)
    common = {
        "lh0t": bf(np.repeat(l0[:, None], LL, axis=1)),
        "ch0t": bf(np.repeat(c0[:, None], CC, axis=1)),
        "id128": bf(np.eye(DIM, dtype=f32)),
        "cu_wiht": bf(inp["Cu_wih"].T), "cu_whht": bf(inp["Cu_whh"].T),
        "lu_wiht_cl": bf(inp["Lu_wih"].T[:DIM]),
        "lu_wiht_fl": bf(inp["Lu_wih"].T[DIM:]),
        "lu_whht": bf(inp["Lu_whh"].T),
        "cu_bias": (inp["Cu_bih"] + inp["Cu_bhh"]).astype(f32).reshape(4, DIM),
        "lu_bias": (inp["Lu_bih"] + inp["Lu_bhh"]).astype(f32).reshape(4, DIM),
    }
    for p, P in (("lmsg", "Lmsg"), ("cmsg", "Cmsg"), ("lvote", "Lvote")):
        for i in (1, 2, 3):
            common[f"{p}_w{i}t"] = bf(inp[f"{P}_w{i}"].T)
            bshape = (1, 1) if (p == "lvote" and i == 3) else (DIM, 1)
            common[f"{p}_b{i}"] = inp[f"{P}_b{i}"].astype(f32).reshape(bshape)
    return [dict(common, b1=b1s[i], b2=b2s[i]) for i in range(N_CORES)]


def kernel(**inputs):
    inp = {k: np.asarray(v) for k, v in inputs.items()}
    in_maps = host_prep(inp)
    nc = _build()
    res = bass_utils.run_bass_kernel_spmd(nc, in_maps,
                                          core_ids=list(range(N_CORES)))
    probs = np.zeros(N_CORES, np.float32)
    for i in range(N_CORES):
        v = res.results[i]["vote"][0]            # [1024]
        s = v[:VPC].astype(np.float64).sum() + \
            v[VPAD:VPAD + VPC].astype(np.float64).sum()
        probs[i] = np.float32(s / (2 * VPC))
    return probs



# revision 24
# speedup vs baseline: 1.1669x; 1.0080x over previous
"""NeuroSAT message-passing GNN on 8 TRN2 NeuronCores (Bass/Tile).

Sharding: clause dim sharded 8-way (2048 padded clauses/core); literal dim
permuted so core i owns problem i's 500 vars (+12 pads) as 1024 lit rows
(512 pos + 512 neg).  Per round (pipelined):
  GEMM2 groups 0,1 -> AllToAll half0 ; groups 2,3 -> AllToAll half1 (fp8)
  partials summed locally on DVE (f32), L-LSTM + L_pre MLP per half,
  AllGather halves of L_pre (fp8, Shared-output Mesh)
  GEMM1 LC.T = L_pre.T @ B1 ; C-LSTM ; C_pre MLP ; repeat
M (counts) is exact in fp8e4m3; fp8 M blocks stream from HBM as the moving
operand against fp8 stationary activations (DoubleRow).  AllToAll is used
instead of ReduceScatter because it always runs the O(1)-hop Mesh algorithm
(RS picks RDH at this size: ~2x slower); the 8 partial blocks are reduced
on the vector engine.  Zero-contribution dummy matmuls keep the PE HAM-warm
(K=8/8 clock) across the residual collective waits.
"""

import numpy as np
import ml_dtypes

import concourse.bass as bass
import concourse.bacc as bacc
import concourse.mybir as mybir
import concourse.tile as tile
from concourse import bass_utils

F32 = mybir.dt.float32
BF16 = mybir.dt.bfloat16
FP8 = mybir.dt.float8e4
AF = mybir.ActivationFunctionType

N_CORES = 8
DIM = 128
N_ROUNDS = 16
N_VARS = 4000
VPC = 500            # real vars per core (= vars per problem)
VPAD = 512           # padded vars per core
LL = 2 * VPAD        # 1024 lit rows per core
LPAD = N_CORES * LL  # 8192
CC = 2048            # padded clauses per core
CPAD = N_CORES * CC  # 16384
KL = LPAD // 128     # 64 k-tiles over lits
KC = CC // 128       # 16 k-tiles over clauses

# GEMM2 groups: group g computes 512-lit chunks J_SETS[g]; chunk j covers
# local lit rows [512*(j%2)...) of destination core j//2.  Groups 0,1 cover
# all even j (RS half 0 = every core's rows 0:512); groups 2,3 odd j.
J_SETS = [[0, 2, 4, 6], [8, 10, 12, 14], [1, 3, 5, 7], [9, 11, 13, 15]]

N_WARM1 = 0          # dummy MMs per gate group, L half 0 (A2A_0 wait)
N_WARM_G1 = 8        # dummy MM prefix on GEMM1 (AG + load window)
N_WARM2 = 0          # dummy MMs per gate group, L half 1 (A2A_1 wait)
N_B2_RES = 8         # resident b2 blocks (of 16); rest streamed per round

nbf = ml_dtypes.bfloat16
nf8 = ml_dtypes.float8_e4m3

_CACHE = {}


def _build():
    """Build + compile the SPMD program once (shape-only, no input values)."""
    if "nc" in _CACHE:
        return _CACHE["nc"]

    nc = bacc.Bacc("TRN2", target_bir_lowering=False, debug=False,
                   num_devices=N_CORES)

    def din(name, shape, dt):
        return nc.dram_tensor(name, shape, dt, kind="ExternalInput")

    # b1: 16 packed groups of 4 k-tiles; rows ordered [half h, core c, r<512]
    b1 = din("b1", [KL // 4, DIM, 4 * CC], FP8)
    # b2[g]: group g's 16 k-tiles packed 4-per-DMA: [4 groups, 4 qgrp, 128, 4*2048]
    b2 = din("b2", [4, 4, DIM, 4 * 2048], FP8)
    lh0t = din("lh0t", [DIM, LL], BF16)
    ch0t = din("ch0t", [DIM, CC], BF16)
    id128 = din("id128", [DIM, DIM], BF16)

    w = {}
    for p in ("lmsg", "cmsg", "lvote"):
        for i in (1, 2, 3):
            shp = [DIM, 1] if (p == "lvote" and i == 3) else [DIM, DIM]
            w[f"{p}_w{i}t"] = din(f"{p}_w{i}t", shp, BF16)
            bshp = [1, 1] if (p == "lvote" and i == 3) else [DIM, 1]
            w[f"{p}_b{i}"] = din(f"{p}_b{i}", bshp, F32)
    w["cu_wiht"] = din("cu_wiht", [DIM, 4 * DIM], BF16)
    w["cu_whht"] = din("cu_whht", [DIM, 4 * DIM], BF16)
    w["lu_wiht_cl"] = din("lu_wiht_cl", [DIM, 4 * DIM], BF16)
    w["lu_wiht_fl"] = din("lu_wiht_fl", [DIM, 4 * DIM], BF16)
    w["lu_whht"] = din("lu_whht", [DIM, 4 * DIM], BF16)
    cu_bias_d = din("cu_bias", [4, DIM], F32)
    lu_bias_d = din("lu_bias", [4, DIM], F32)

    vote_out = nc.dram_tensor("vote", [1, LL], F32, kind="ExternalOutput")

    with tile.TileContext(nc) as tc, \
         tc.tile_pool(name="const", bufs=1) as const, \
         tc.tile_pool(name="sb", bufs=2) as sb, \
         tc.tile_pool(name="sb3", bufs=2) as sb3, \
         tc.tile_pool(name="ps", bufs=6, space="PSUM") as ps, \
         tc.tile_pool(name="pstr", bufs=2, space="PSUM") as pstr, \
         tc.tile_pool(name="dram", bufs=2, space="DRAM") as dram:

        # ---- resident b2: first N_B2_RES of 16 blocks of [128, 8192] fp8
        b2r = const.tile([DIM, N_B2_RES * 4 * 2048], FP8, tag="b2r")
        for g in range(4):
            for q in range(4):
                if g * 4 + q >= N_B2_RES:
                    continue
                sl = slice((g * 4 + q) * 8192, (g * 4 + q + 1) * 8192)
                nc.sync.dma_start(b2r[:, sl], b2.ap()[g, q, :, :])

        # ---- load constants/weights into SBUF
        cw = {}
        for k in w:
            t = const.tile(list(w[k].shape), w[k].dtype, tag=f"cw_{k}")
            nc.sync.dma_start(t[:], w[k].ap())
            cw[k] = t
        for k, dte in (("cu_bias", cu_bias_d), ("lu_bias", lu_bias_d)):
            t = const.tile([DIM, 4], F32, tag=f"cw_{k}")
            nc.sync.dma_start(t[:], dte.ap().rearrange("g p -> p g"))
            cw[k] = t
        idt = const.tile([DIM, DIM], BF16, tag="idt")
        nc.sync.dma_start(idt[:], id128.ap())
        zbf = const.tile([DIM, 512], BF16, tag="zbf")
        nc.vector.memset(zbf[:], 0.0)

        # ---- persistent state (feature-major)
        lht = const.tile([DIM, LL], BF16, tag="lht")
        lct = const.tile([DIM, LL], F32, tag="lct")
        cht = const.tile([DIM, CC], BF16, tag="cht")
        cct = const.tile([DIM, CC], F32, tag="cct")
        nc.sync.dma_start(lht[:], lh0t.ap())
        nc.sync.dma_start(cht[:], ch0t.ap())
        nc.vector.memset(lct[:], 0.0)
        nc.vector.memset(cct[:], 0.0)

        def dma2(dst, src):
            """Split a [128, N] transfer across two DMA queues by partitions."""
            nc.sync.dma_start(dst[0:64, :], src[0:64, :])
            nc.sync.dma_start(dst[64:DIM, :], src[64:DIM, :])

        def mlp_chunk(x, pfx, sl, n, out_dt=BF16, tagsfx=""):
            """3-layer MLP on columns sl (chunks of <=512) of x [128, *]."""
            cur = x
            for li in (1, 2, 3):
                wt = cw[f"{pfx}_w{li}t"]
                bt = cw[f"{pfx}_b{li}"]
                m = wt.shape[1]
                o = sb.tile([m, n], out_dt if li == 3 else BF16, bufs=1,
                            tag=f"{pfx}_h{li}{tagsfx}", name=f"{pfx}_h{li}{tagsfx}")
                for rc in range(n // 512):
                    c0 = rc * 512
                    pt = ps.tile([m, 512], F32, tag="ps", name="mlp_ps")
                    src = cur[:, sl.start + c0:sl.start + c0 + 512] if li == 1 \
                        else cur[:, c0:c0 + 512]
                    nc.tensor.matmul(pt[:], wt[:], src, start=True, stop=True)
                    func = AF.Relu if li < 3 else AF.Identity
                    nc.scalar.activation(o[:, c0:c0 + 512], pt[:], func,
                                         bias=bt[:, 0:1])
                cur = o
            return cur

        def lstm_elementwise(gps, bias, c_st, h_st, rc0, n):
            """gps: 4 psum tiles [128, n] (i,f,g,o); updates states [:, rc0:rc0+n]."""
            sl = slice(rc0, rc0 + n)
            sig_i = sb.tile([DIM, n], BF16, tag="lw_si", bufs=1, name="sig_i")
            sig_f = sb.tile([DIM, n], BF16, tag="lw_sf", bufs=1, name="sig_f")
            tng = sb.tile([DIM, n], BF16, tag="lw_tg", bufs=1, name="tng")
            sig_o = sb.tile([DIM, n], BF16, tag="lw_so", bufs=1, name="sig_o")
            nc.scalar.activation(sig_i[:], gps[0][:], AF.Sigmoid, bias=bias[:, 0:1])
            nc.scalar.activation(sig_f[:], gps[1][:], AF.Sigmoid, bias=bias[:, 1:2])
            nc.scalar.activation(tng[:], gps[2][:], AF.Tanh, bias=bias[:, 2:3])
            nc.scalar.activation(sig_o[:], gps[3][:], AF.Sigmoid, bias=bias[:, 3:4])
            t1 = sb.tile([DIM, n], F32, tag="lw_t1", bufs=1, name="t1")
            nc.vector.tensor_mul(t1[:], sig_f[:], c_st[:, sl])
            t2 = sb.tile([DIM, n], F32, tag="lw_t2", bufs=1, name="t2")
            nc.vector.tensor_mul(t2[:], sig_i[:], tng[:])
            nc.vector.tensor_add(c_st[:, sl], t1[:], t2[:])
            tnc = sb.tile([DIM, n], BF16, tag="lw_tc", bufs=1, name="tnc")
            nc.scalar.activation(tnc[:], c_st[:, sl], AF.Tanh)
            nc.vector.tensor_mul(h_st[:, sl], sig_o[:], tnc[:])

        def c_phase(lct_ps):
            """C-LSTM + C_pre MLP + transposes -> cpre_kt [128, 16*128]."""
            for rc in range(4):
                sl = slice(rc * 512, (rc + 1) * 512)
                lc_sb = sb.tile([DIM, 512], BF16, tag="lc_sb", bufs=2,
                                name=f"lc_sb{rc}")
                nc.vector.tensor_copy(lc_sb[:], lct_ps[rc][:])
                gps = [ps.tile([DIM, 512], F32, tag="ps", name=f"cg{i}")
                       for i in range(4)]
                for g in range(4):
                    gsl = slice(g * DIM, (g + 1) * DIM)
                    nc.tensor.matmul(gps[g][:], cw["cu_wiht"][:, gsl],
                                     lc_sb[:], start=True, stop=False)
                    nc.tensor.matmul(gps[g][:], cw["cu_whht"][:, gsl],
                                     cht[:, sl], start=False, stop=True)
                lstm_elementwise(gps, cw["cu_bias"], cct, cht, rc * 512, 512)
            cpre_kt = sb.tile([DIM, KC * DIM], FP8, tag="cpre_kt", bufs=1,
                              name="cpre_kt")
            for ch in range(2):
                cpreT = mlp_chunk(cht, "cmsg", slice(ch * 1024, (ch + 1) * 1024),
                                  1024)
                for t in range(8):
                    sl = slice(t * DIM, (t + 1) * DIM)
                    osl = slice((ch * 8 + t) * DIM, (ch * 8 + t + 1) * DIM)
                    pt = pstr.tile([DIM, DIM], BF16, tag="pstr", name="cp_tr")
                    nc.tensor.transpose(pt[:], cpreT[:, sl], idt[:])
                    nc.vector.tensor_copy(cpre_kt[:, osl], pt[:])
            return cpre_kt

        N_B2_PRE = 3     # streamed-b2 window depth

        def b2_fetch(i):
            t = sb3.tile([DIM, 4 * 2048], FP8, tag="b2t", bufs=N_B2_PRE,
                         name=f"b2t{i}")
            nc.scalar.dma_start(t[:], b2.ap()[i // 4, i % 4, :, :])
            return t

        def gemm2_group(cpre_kt, g, rs_bufs, r, b2s):
            """One GEMM2 group: 4 psum accums over KC k-tiles; stage to RS buf."""
            cl_ps = [ps.tile([DIM, 512], F32, tag="ps", name=f"cl{g}_{i}")
                     for i in range(4)]
            for q in range(4):
                blk = g * 4 + q
                if blk >= N_B2_RES:
                    b2t = b2s[blk - N_B2_RES]
                    b2v = b2t[:].rearrange("p (t c) -> p t c", c=2048)
                else:
                    gsl = slice(blk * 8192, (blk + 1) * 8192)
                    b2v = b2r[:, gsl].rearrange("p (t c) -> p t c", c=2048)
                for kk in (0, 2):
                    k = 4 * q + kk
                    ck = cpre_kt[:, k * DIM:(k + 2) * DIM].rearrange(
                        "p (j d) -> p j d", j=2)
                    for i in range(4):
                        nc.tensor.matmul(
                            cl_ps[i][:], ck,
                            b2v[:, kk:kk + 2, i * 512:(i + 1) * 512],
                            start=(k == 0), stop=(k == KC - 2),
                            perf_mode=mybir.MatmulPerfMode.DoubleRow)
                if blk >= N_B2_RES and blk + N_B2_PRE < 16:
                    b2s.append(b2_fetch(blk + N_B2_PRE))
            for i in range(4):
                j = J_SETS[g][i]
                h, blk = j % 2, j // 2
                cs = sb.tile([DIM, 512], FP8, tag="cl_st", name="cl_st")
                nc.vector.tensor_copy(cs[:], cl_ps[i][:])
                dma2(rs_bufs[h][blk * DIM:(blk + 1) * DIM, :], cs[:])

        def l_half(h, clt_h, lh_flip, r, n_warm, ag_in):
            """L-LSTM + L_pre MLP + transposes for local half h; returns ag_in."""
            sl = slice(h * 512, (h + 1) * 512)
            fsl = slice((1 - h) * 512, (2 - h) * 512)
            gps = [ps.tile([DIM, 512], F32, tag="ps", name=f"lg{h}_{i}")
                   for i in range(4)]
            # flip/hidden gate matmuls first: they only need lh_flip, so the
            # PE computes them while the A2A exchange is still in flight; the
            # clt matmuls (which wait on the reduce) come last.
            for g in range(4):
                gsl = slice(g * DIM, (g + 1) * DIM)
                for wi in range(n_warm):
                    nc.tensor.matmul(gps[g][:], idt[:], zbf[:],
                                     start=(wi == 0), stop=False)
                nc.tensor.matmul(gps[g][:], cw["lu_wiht_fl"][:, gsl],
                                 lh_flip[:, fsl], start=(n_warm == 0),
                                 stop=False)
                nc.tensor.matmul(gps[g][:], cw["lu_whht"][:, gsl],
                                 lh_flip[:, sl], start=False, stop=False)
            for g in range(4):
                gsl = slice(g * DIM, (g + 1) * DIM)
                nc.tensor.matmul(gps[g][:], cw["lu_wiht_cl"][:, gsl],
                                 clt_h[:], start=False, stop=True)
            lstm_elementwise(gps, cw["lu_bias"], lct, lht, h * 512, 512)
            lpre_h = mlp_chunk(lht, "lmsg", sl, 512, tagsfx=f"_{h}")
            for t in range(4):
                tsl = slice(t * DIM, (t + 1) * DIM)
                pt = pstr.tile([DIM, DIM], BF16, tag="pstr", name="lp_tr")
                nc.tensor.transpose(pt[:], lpre_h[:, tsl], idt[:])
                st = sb.tile([DIM, DIM], FP8, tag="tr_st", name="tr_st")
                nc.vector.tensor_copy(st[:], pt[:])
                nc.sync.dma_start(ag_in[tsl, :], st[:])

        N_B1_PRE = 4     # b1 window depth (tile bufs / prologue prefetch)

        def b1_fetch(grp):
            t = sb3.tile([DIM, 4 * CC], FP8, tag="b1t", bufs=N_B1_PRE,
                         name=f"b1t{grp}")
            nc.scalar.dma_start(t[:], b1.ap()[grp, :, :])
            return t

        def gemm1_prologue():
            """Prefetch the first b1 groups; fires as the prior GEMM1 ends."""
            return [b1_fetch(j) for j in range(N_B1_PRE)]

        def gemm1(lpre_sb, n_warm, pre):
            """GEMM1: LC.T [128, 2048] psum accums over 64 packed k-tiles.

            Each group's refill DMA is issued right after the matmuls that
            free its buffer slot, so the scalar HWDGE ring never stalls."""
            tiles = list(pre)
            lct_ps = [ps.tile([DIM, 512], F32, tag="ps", name=f"g1_{i}")
                      for i in range(4)]
            for wi in range(n_warm):
                nc.tensor.matmul(lct_ps[wi % 4][:], idt[:], zbf[:],
                                 start=(wi < 4), stop=False)
            for grp in range(KL // 4):
                b1t = tiles[grp]
                b1v = b1t[:].rearrange("p (t c) -> p t c", c=CC)
                for kk in (0, 2):
                    k = 4 * grp + kk
                    lf = lpre_sb[k // 8]
                    t0 = k % 8
                    lk = lf[:, t0 * DIM:(t0 + 2) * DIM].rearrange(
                        "p (j d) -> p j d", j=2)
                    for c4 in range(4):
                        nc.tensor.matmul(
                            lct_ps[c4][:], lk,
                            b1v[:, kk:kk + 2, c4 * 512:(c4 + 1) * 512],
                            start=(k == 0 and n_warm == 0),
                            stop=(k == KL - 2),
                            perf_mode=mybir.MatmulPerfMode.DoubleRow)
                if grp + N_B1_PRE < KL // 4:
                    tiles.append(b1_fetch(grp + N_B1_PRE))
            return lct_ps

        def load_lpre(ag_outs):
            """Load AG halves as 8 SBUF chunks of 8 k-tiles each."""
            lpre_sb = []
            for c8 in range(8):
                lt = sb.tile([DIM, 8 * DIM], FP8, tag="lpf", bufs=6,
                             name=f"lpf{c8}")
                src = ag_outs[c8 // 4][(c8 % 4) * 1024:(c8 % 4 + 1) * 1024, :]
                s3 = src.rearrange("(t p) d -> p t d", p=DIM)
                d3 = lt[:].rearrange("p (t d) -> p t d", d=DIM)
                nc.sync.dma_start(d3[0:64], s3[0:64])
                nc.sync.dma_start(d3[64:DIM], s3[64:DIM])
                lpre_sb.append(lt)
            return lpre_sb

        rg = [list(range(N_CORES))]

        def collective(kind, op, cin, cout):
            nc.gpsimd.collective_compute(kind, op, replica_groups=rg,
                                         ins=[cin.opt()], outs=[cout.opt()])

        # ====== round 0 head: L_pre from Lh0 -> ag_in halves ======
        ag_ins = []
        for h in range(2):
            ag_in = dram.tile([512, DIM], FP8, tag=f"ag_in{h}",
                              name=f"ag_in{h}_init")
            lpre_h = mlp_chunk(lht, "lmsg", slice(h * 512, (h + 1) * 512),
                               512, tagsfx=f"_{h}")
            for t in range(4):
                tsl = slice(t * DIM, (t + 1) * DIM)
                pt = pstr.tile([DIM, DIM], BF16, tag="pstr", name="lp_tr0")
                nc.tensor.transpose(pt[:], lpre_h[:, tsl], idt[:])
                st = sb.tile([DIM, DIM], FP8, tag="tr_st", name="tr_st0")
                nc.vector.tensor_copy(st[:], pt[:])
                nc.sync.dma_start(ag_in[tsl, :], st[:])
            ag_ins.append(ag_in)

        def a2a_reduce(h, ro, r):
            """Load A2A output (8 partial blocks) and tree-sum -> clt bf16.

            Stage-1 sums of two fp8 values are exact in bf16; the 3-level
            tree keeps the DVE ops independent (no serial RAW chain)."""
            a2a_sb = sb.tile([DIM, 8 * 512], FP8, tag="a2a_sb", bufs=2,
                             name=f"a2a_sb{h}_{r}")
            src3 = ro[:].rearrange("(b p) c -> p b c", p=DIM)
            dst3 = a2a_sb[:].rearrange("p (b c) -> p b c", c=512)
            nc.sync.dma_start(dst3[0:64], src3[0:64])
            nc.sync.dma_start(dst3[64:DIM], src3[64:DIM])
            blk = lambda b: a2a_sb[:, b * 512:(b + 1) * 512]
            s1 = sb.tile([DIM, 4 * 512], BF16, tag="a2a_s1", bufs=1,
                         name=f"s1_{h}_{r}")
            for b in range(4):
                nc.vector.tensor_add(s1[:, b * 512:(b + 1) * 512],
                                     blk(2 * b), blk(2 * b + 1))
            s2 = sb.tile([DIM, 2 * 512], F32, tag="a2a_s2", bufs=1,
                         name=f"s2_{h}_{r}")
            for b in range(2):
                nc.vector.tensor_add(s2[:, b * 512:(b + 1) * 512],
                                     s1[:, 2 * b * 512:(2 * b + 1) * 512],
                                     s1[:, (2 * b + 1) * 512:(2 * b + 2) * 512])
            clt = sb.tile([DIM, 512], BF16, tag=f"clt{h}", bufs=1,
                          name=f"clt{h}_{r}")
            nc.vector.tensor_add(clt[:], s2[:, 0:512], s2[:, 512:1024])
            return clt

        b1pre = gemm1_prologue()
        for r in range(N_ROUNDS):
            ag_outs = []
            for h in range(2):
                ag_out = dram.tile([4096, DIM], FP8, tag=f"ag_out{h}",
                                   addr_space="Shared", name=f"ag_out{h}_{r}")
                collective("AllGather", mybir.AluOpType.bypass,
                           ag_ins[h], ag_out)
                ag_outs.append(ag_out)
            lpre_sb = load_lpre(ag_outs)
            lct_ps = gemm1(lpre_sb, N_WARM_G1, b1pre)
            if r + 1 < N_ROUNDS:
                b1pre = gemm1_prologue()
            b2s = [b2_fetch(i) for i in
                   range(N_B2_RES, min(16, N_B2_RES + N_B2_PRE))]

            cpre_kt = c_phase(lct_ps)

            lh_flip = sb.tile([DIM, LL], BF16, tag="lh_flip", bufs=1,
                              name="lh_flip")
            nc.vector.tensor_copy(lh_flip[:], lht[:])

            rs_bufs = [dram.tile([N_CORES * DIM, 512], FP8, tag=f"rs_in{h}",
                                 name=f"rs_in{h}_{r}") for h in range(2)]
            gemm2_group(cpre_kt, 0, rs_bufs, r, b2s)
            gemm2_group(cpre_kt, 1, rs_bufs, r, b2s)
            ro0 = dram.tile([N_CORES * DIM, 512], FP8, tag="rs_out0",
                            name=f"rs_out0_{r}")
            collective("AllToAll", mybir.AluOpType.bypass, rs_bufs[0], ro0)
            gemm2_group(cpre_kt, 2, rs_bufs, r, b2s)
            gemm2_group(cpre_kt, 3, rs_bufs, r, b2s)

            ro1 = dram.tile([N_CORES * DIM, 512], FP8, tag="rs_out1",
                            name=f"rs_out1_{r}")
            collective("AllToAll", mybir.AluOpType.bypass, rs_bufs[1], ro1)

            ag_ins = [dram.tile([512, DIM], FP8, tag=f"ag_in{h}",
                                name=f"ag_in{h}_{r}") for h in range(2)]
            clt0 = a2a_reduce(0, ro0, r)
            l_half(0, clt0, lh_flip, r, N_WARM1, ag_ins[0])

            clt1 = a2a_reduce(1, ro1, r)
            l_half(1, clt1, lh_flip, r, N_WARM2, ag_ins[1])

        # ---- vote MLP on final Lh -> [1, 1024] f32
        vt0 = mlp_chunk(lht, "lvote", slice(0, 512), 512, out_dt=F32,
                        tagsfx="_0")
        vt1 = mlp_chunk(lht, "lvote", slice(512, 1024), 512, out_dt=F32,
                        tagsfx="_1")
        nc.sync.dma_start(vote_out.ap()[:, 0:512], vt0[:])
        nc.sync.dma_start(vote_out.ap()[:, 512:1024], vt1[:])

    nc.compile()
    _CACHE["nc"] = nc
    return nc


def _perm_rows(lits):
    """Map global lit index -> permuted row (core-major, 1024 rows/core)."""
    lits = np.asarray(lits)
    neg = lits >= N_VARS
    v = np.where(neg, lits - N_VARS, lits)
    core = v // VPC
    r = v % VPC
    return core * LL + np.where(neg, VPAD + r, r)


def _b1_row_order():
    """B1 rows: [half h, core c, r] -> permuted row c*1024 + h*512 + r."""
    order = np.empty(LPAD, np.int64)
    n = 0
    for h in range(2):
        for c in range(N_CORES):
            order[n:n + 512] = c * LL + h * 512 + np.arange(512)
            n += 512
    return order


def host_prep(inp):
    f32 = np.float32
    idx = inp["L_unpack_indices"].astype(np.int64)
    rows = _perm_rows(idx[:, 0])
    M = np.zeros((LPAD, CPAD), np.float32)
    np.add.at(M, (rows, idx[:, 1]), 1.0)

    row_order = _b1_row_order()
    b1s, b2s = [], []
    for i in range(N_CORES):
        blk = M[:, i * CC:(i + 1) * CC]          # [8192, 2048] permuted rows
        b1o = blk[row_order]                      # AG-concat row order
        # pack 4 k-tiles per DMA group: [16, 128, 4*2048]
        b1p = b1o.reshape(16, 4, DIM, CC).transpose(0, 2, 1, 3) \
                 .reshape(16, DIM, 4 * CC)
        b1s.append(np.ascontiguousarray(b1p).astype(nf8))
        bT = blk.T                                # [2048 clauses, 8192 lits]
        grp = []
        for g in range(4):
            cols = np.concatenate([np.arange(j * 512, (j + 1) * 512)
                                   for j in J_SETS[g]])
            gb = bT[:, cols]                      # [2048, 2048]
            gp = gb.reshape(4, 4, DIM, 2048).transpose(0, 2, 1, 3) \
                   .reshape(4, DIM, 4 * 2048)
            grp.append(gp)
        b2s.append(np.ascontiguousarray(np.stack(grp)).astype(nf8))

    def bf(x):
        return np.ascontiguousarray(x).astype(nbf)

    l0 = (inp["L_init_w"][:, 0] + inp["L_init_b"]).astype(f32)
    c0 = (inp["C_init_w"][:, 0] + inp["C_init_b"]).astype(f32)
    common = {
        "lh0t": bf(np.repeat(l0[:, None], LL, axis=1)),
        "ch0t": bf(np.repeat(c0[:, None], CC, axis=1)),
        "id128": bf(np.eye(DIM, dtype=f32)),
        "cu_wiht": bf(inp["Cu_wih"].T), "cu_whht": bf(inp["Cu_whh"].T),
        "lu_wiht_cl": bf(inp["Lu_wih"].T[:DIM]),
        "lu_wiht_fl": bf(inp["Lu_wih"].T[DIM:]),
        "lu_whht": bf(inp["Lu_whh"].T),
        "cu_bias": (inp["Cu_bih"] + inp["Cu_bhh"]).astype(f32).reshape(4, DIM),
        "lu_bias": (inp["Lu_bih"] + inp["Lu_bhh"]).astype(f32).reshape(4, DIM),
    }
    for p, P in (("lmsg", "Lmsg"), ("cmsg", "Cmsg"), ("lvote", "Lvote")):
        for i in (1, 2, 3):
            common[f"{p}_w{i}t"] = bf(inp[f"{P}_w{i}"].T)
            bshape = (1, 1) if (p == "lvote" and i == 3) else (DIM, 1)
            common[f"{p}_b{i}"] = inp[f"{P}_b{i}"].astype(f32).reshape(bshape)
    return [dict(common, b1=b1s[i], b2=b2s[i]) for i in range(N_CORES)]


def kernel(**inputs):
    inp = {k: np.asarray(v) for k, v in inputs.items()}
    in_maps = host_prep(inp)
    nc = _build()
    res = bass_utils.run_bass_kernel_spmd(nc, in_maps,
                                          core_ids=list(range(N_CORES)))
    probs = np.zeros(N_CORES, np.float32)
    for i in range(N_CORES):
        v = res.results[i]["vote"][0]            # [1024]
        s = v[:VPC].astype(np.float64).sum() + \
            v[VPAD:VPAD + VPC].astype(np.float64).sum()
        probs[i] = np.float32(s / (2 * VPC))
    return probs

